# revision 1
# baseline (speedup 1.0000x reference)
"""Trainium2 Bass kernel: 16-head MHA (B=2, S=2048, E=1024) on 8 NeuronCores.

Sharding: core c = (batch b = c // 4, head-group g = c % 4); each core runs
4 heads of one batch (data parallel on B x tensor parallel on heads).  The
output projection is row-sharded: each core produces a partial [S, E] f32
output; the host sums the 4 head-group partials per batch and adds bo.

Device pipeline per core (all matmul operands bf16, fp32 PSUM accumulation):
  qT[d,m] = WqT.T-contract-e(xqT)     (weight-tile stationary, reused over
                                       all 4 m-chunks before switching)
  kT[d,n] = same
  vT[dv,n] = like q/k, then PE-transposed 128x128 into v_aug [n, dv] tiles
            carrying an extra ones column per head so the attention matmul
            also yields the softmax denominators
  scoresT[n,m] = kT-tile stationary (K=64; the two heads of a pair run in
            different PE row groups concurrently), qT moving; fully-masked
            causal columns are never computed (restricted matmul widths)
  probsT = exp(scoresT / sqrt(dk)) via one ACT per (j, head-pair); 0/1
            triangle multiply on diagonal tiles only (gpsimd)
  o_aug[dv+1,m] = v_aug stationary, probsT moving, accumulated over n-tiles,
            software-pipelined two steps behind the scores matmuls; row 64
            is sum(probs) = softmax denominator
  oT = o_aug[0:64] * bcast(1/denom): denom bounced through DRAM, re-read
            reshaped [128,4] so the reciprocal runs 128 lanes wide, bounced
            back, re-read with a step-0 partition AP as a [64,MC] broadcast
  out[m,e] partial = oT-tile stationary (serves both e-chunks), WoT moving
"""

import numpy as np
import ml_dtypes

B, S, E = 2, 2048, 1024
H, DK = 16, 64
NCORES = 8
G = 4                 # head-groups (tensor parallel degree)
NH = H // G           # heads per core = 4
DKH = NH * DK         # 256 head dims per core
P = 128
MC = 512              # m-chunk (psum bank width in f32)
NMC = S // MC         # 4 m-chunks
NT = S // P           # 16 n-tiles (and m-tiles)
ET = E // P           # 8 e-tiles
PAIRS = NH // 2       # 2 head pairs per core
BF16 = ml_dtypes.bfloat16
SCALE = float(1.0 / np.sqrt(np.float32(DK)))


def _build_program(chunk_ntiles, causal, bias_qk, bias_v):
    """Build the (SPMD, shared across all 8 cores) Bass program.

    chunk_ntiles[c] = number of 128-wide n-tiles to process for m-chunk c.
    causal: apply diagonal-tile masking (memset + tri multiply).
    """
    from contextlib import ExitStack

    import concourse.bass as bass
    import concourse.tile as tile
    from concourse import bacc, mybir
    from concourse.masks import make_identity

    f32 = mybir.dt.float32
    bf16 = mybir.dt.bfloat16
    Exp = mybir.ActivationFunctionType.Exp

    nc = bacc.Bacc(
        "TRN2",
        target_bir_lowering=False,
        debug=False,
        enable_asserts=False,
        num_devices=NCORES,
    )

    # ---- DRAM I/O ----
    xqT = nc.dram_tensor("xqT", [E, S], bf16, kind="ExternalInput").ap()
    xkT = nc.dram_tensor("xkT", [E, S], bf16, kind="ExternalInput").ap()
    xvT = nc.dram_tensor("xvT", [E, S], bf16, kind="ExternalInput").ap()
    wkqvT = nc.dram_tensor("wkqvT", [E, 3 * DKH], bf16, kind="ExternalInput").ap()
    woT = nc.dram_tensor("woT", [DKH, E], bf16, kind="ExternalInput").ap()
    dmask = nc.dram_tensor("dmask", [P, P], bf16, kind="ExternalInput").ap()
    if bias_qk:
        bqd = nc.dram_tensor("bq", [DKH, 1], f32, kind="ExternalInput").ap()
        bkd = nc.dram_tensor("bk", [DKH, 1], f32, kind="ExternalInput").ap()
    if bias_v:
        bvd = nc.dram_tensor("bv", [DKH, 1], f32, kind="ExternalInput").ap()
    out = nc.dram_tensor("out", [S, E], f32, kind="ExternalOutput").ap()

    with tile.TileContext(nc) as tc, ExitStack() as ctx:
        const = ctx.enter_context(tc.tile_pool(name="const", bufs=1))
        xpool = ctx.enter_context(tc.tile_pool(name="xpool", bufs=1))
        wpool = ctx.enter_context(tc.tile_pool(name="wpool", bufs=1))
        qkpool = ctx.enter_context(tc.tile_pool(name="qkpool", bufs=1))
        vpool = ctx.enter_context(tc.tile_pool(name="vpool", bufs=1))
        prpool = ctx.enter_context(tc.tile_pool(name="prpool", bufs=10))
        rcpool = ctx.enter_context(tc.tile_pool(name="rcpool", bufs=4))
        otpool = ctx.enter_context(tc.tile_pool(name="otpool", bufs=1))
        ostpool = ctx.enter_context(tc.tile_pool(name="ostpool", bufs=4))

        # weight tiles: wkqv packed on host into one [E, 3*DKH] tensor; one
        # DMA per 128-row block ([128, 768] = 1.5KB/partition contiguous).
        # DMA *issue* costs ~0.6us on the issuing engine regardless of size,
        # so few+large transfers matter more than fine-grained pacing.  The
        # first e-tile goes out first on the sync queue so the very first
        # projection LDWEIGHTS unblocks at ~1us.
        wkqv_sb = wpool.tile([P, ET, 3 * DKH], bf16, tag="wkqv")
        for i in range(ET):
            eng = nc.sync if i == 0 else nc.scalar
            eng.dma_start(out=wkqv_sb[:, i, :], in_=wkqvT[P * i : P * (i + 1), :])
        wk_sb = wkqv_sb[:, :, 0:DKH]
        wq_sb = wkqv_sb[:, :, DKH : 2 * DKH]
        wv_sb = wkqv_sb[:, :, 2 * DKH : 3 * DKH]
        wo_sb = wpool.tile([P, PAIRS, E], bf16, tag="wo")
        for p in range(PAIRS):
            nc.scalar.dma_start(out=wo_sb[:, p, :], in_=woT[P * p : P * (p + 1), :])

        if bias_qk:
            bq_sb = const.tile([P, PAIRS], f32, tag="bq")
            nc.sync.dma_start(out=bq_sb, in_=bqd.rearrange("(t p) o -> p (t o)", p=P))
            bk_sb = const.tile([P, PAIRS], f32, tag="bk")
            nc.sync.dma_start(out=bk_sb, in_=bkd.rearrange("(t p) o -> p (t o)", p=P))
        if bias_v:
            bv_sb = const.tile([P, PAIRS], f32, tag="bv")
            nc.sync.dma_start(out=bv_sb, in_=bvd.rearrange("(t p) o -> p (t o)", p=P))

        # x inputs: four 1MB DMAs per tensor (two e-tiles each, 4KB bursts) —
        # DMA issue costs ~0.6us/transfer on the issuing engine, so use few
        # large transfers, one tensor per DMA-capable engine in parallel
        xk_sb, xq_sb, xv_sb = [], [], []
        for x_sb, xT, engs, nm in (
            (xk_sb, xkT, (nc.sync,) * 4, "xk"),
            (xq_sb, xqT, (nc.scalar,) * 4, "xq"),
            (xv_sb, xvT, (nc.sync, nc.scalar, nc.sync, nc.scalar), "xv"),
        ):
            for i2 in range(ET // 2):
                pair = xpool.tile(
                    [P, 2, S], bf16, tag=f"{nm}pr{i2}", name=f"{nm}pair{i2}"
                )
                src = xT.rearrange("(t p) s -> p t s", p=P)[:, 2 * i2 : 2 * i2 + 2, :]
                if nm == "xk" and i2 == 0:
                    # split the first transfer so the first matmuls of the
                    # k-projection unblock after half the bytes
                    engs[i2].dma_start(out=pair[:, 0, :], in_=src[:, 0, :])
                    engs[i2].dma_start(out=pair[:, 1, :], in_=src[:, 1, :])
                else:
                    engs[i2].dma_start(out=pair, in_=src)
                x_sb.append(pair[:, 0, :])
                x_sb.append(pair[:, 1, :])

        # diagonal-mask constant: only needed once attention starts
        dmask_sb = const.tile([P, P], bf16, tag="dmask")
        nc.scalar.dma_start(out=dmask_sb, in_=dmask)

        # persistent activation tiles
        qT_sb = [qkpool.tile([P, S], bf16, tag=f"qT{p}", name=f"qT_sb{p}") for p in range(PAIRS)]
        kT_sb = [qkpool.tile([P, S], bf16, tag=f"kT{p}", name=f"kT_sb{p}") for p in range(PAIRS)]
        vaug_sb = [vpool.tile([P, NH, DK + 1], bf16, tag=f"va{j}", name=f"vaug_sb{j}") for j in range(NT)]
        oT_sb = [otpool.tile([P, S], bf16, tag=f"oT{p}", name=f"oT_sb{p}") for p in range(PAIRS)]

        # ---- stage 1: projections ----
        # q/k: weight-tile stationary, streamed over all 4 chunks (psum x4)
        # v:   x-tile stationary split into two row-group halves (concurrent)
        with tc.tile_pool(name="pj_ps", bufs=4, space="PSUM") as pjps:
            for dst, w_sb, x_sb, bias in (
                (kT_sb, wk_sb, xk_sb, bk_sb if bias_qk else None),
                (qT_sb, wq_sb, xq_sb, bq_sb if bias_qk else None),
            ):
                for p in range(PAIRS):
                    ps = [pjps.tile([P, MC], f32, tag="qk", name="ps_qk") for _ in range(NMC)]
                    for i in range(ET):
                        for c in range(NMC):
                            nc.tensor.matmul(
                                ps[c],
                                w_sb[:, i, P * p : P * (p + 1)],
                                x_sb[i][:, MC * c : MC * (c + 1)],
                                start=(i == 0),
                                stop=(i == ET - 1),
                            )
                    for c in range(NMC):
                        dslice = dst[p][:, MC * c : MC * (c + 1)]
                        if bias is not None:
                            nc.vector.tensor_scalar_add(dslice, ps[c], bias[:, p : p + 1])
                        else:
                            nc.scalar.copy(dslice, ps[c])
            # v: weight-block stationary producing vT [dv, n] (same efficient
            # shape as q/k), then PE-transpose 128x128 tiles into v_aug [n, dv]
            with tc.tile_pool(name="tp_ps", bufs=2, space="PSUM") as tpps:
                ident = const.tile([P, P], bf16, tag="ident")
                make_identity(nc, ident)
                vT_sb = [
                    qkpool.tile([P, S], bf16, tag=f"vT{db}", name=f"vT_sb{db}")
                    for db in range(PAIRS)
                ]
                for db in range(PAIRS):
                    ps = [pjps.tile([P, MC], f32, tag="qk", name="ps_v") for _ in range(NMC)]
                    for i in range(ET):
                        for c in range(NMC):
                            nc.tensor.matmul(
                                ps[c],
                                wv_sb[:, i, P * db : P * (db + 1)],
                                xv_sb[i][:, MC * c : MC * (c + 1)],
                                start=(i == 0),
                                stop=(i == ET - 1),
                            )
                    for c in range(NMC):
                        vslice = vT_sb[db][:, MC * c : MC * (c + 1)]
                        if bias_v:
                            nc.vector.tensor_scalar_add(vslice, ps[c], bv_sb[:, db : db + 1])
                        else:
                            nc.vector.tensor_copy(vslice, ps[c])
                    for j in range(NT):
                        pt = tpps.tile([P, P], bf16, tag="pt", name="pt_t")
                        nc.tensor.transpose(pt, vT_sb[db][:, P * j : P * (j + 1)], ident)
                        pt3 = pt.rearrange("n (h d) -> n h d", h=2)
                        nc.vector.tensor_copy(vaug_sb[j][:, 2 * db : 2 * db + 2, 0:DK], pt3)
                        if db == PAIRS - 1:
                            nc.vector.memset(vaug_sb[j][:, :, DK : DK + 1], 1.0)

        # ---- stage 2+3: attention with interleaved output projection ----
        # The oaps PSUM pool is shared between o_aug accumulators and output-
        # projection tiles (same shape) so 8 banks suffice while the outproj
        # matmuls fill the PE during the softmax epilogues.
        with (
            tc.tile_pool(name="sc_ps", bufs=2, space="PSUM") as scps,
            tc.tile_pool(name="oa_ps", bufs=4, space="PSUM") as oaps,
            tc.tile_pool(name="rc_dram", bufs=4, space="DRAM") as rcdram,
        ):
            for c in range(NMC):
                J = chunk_ntiles[c]
                for p in range(PAIRS):
                    oaug = [
                        oaps.tile([P, MC], f32, tag="oaug", name=f"oaug{h01}")
                        for h01 in range(2)
                    ]
                    probs_tiles = [None] * J

                    def scores_step(j):
                        # columns left of `off` in this m-chunk are fully
                        # masked for n-tile j: never compute/exp/consume them
                        off = P * (j - 4 * c) if (causal and j >= 4 * c) else 0
                        sc = scps.tile([P, 2 * MC], f32, tag="sc", name="sc_ps_t")
                        for h01 in range(2):
                            nc.tensor.matmul(
                                sc[:, MC * h01 + off : MC * (h01 + 1)],
                                kT_sb[p][64 * h01 : 64 * (h01 + 1), P * j : P * (j + 1)],
                                qT_sb[p][64 * h01 : 64 * (h01 + 1), MC * c + off : MC * (c + 1)],
                                start=True,
                                stop=True,
                            )
                        probs = prpool.tile([P, 2 * MC], bf16, tag="probs", name="probs_t")
                        sc3 = sc.rearrange("p (u m) -> p u m", u=2)
                        pr3 = probs.rearrange("p (u m) -> p u m", u=2)
                        nc.scalar.activation(
                            pr3[:, :, off:MC], sc3[:, :, off:MC], Exp, bias=0.0, scale=SCALE
                        )
                        if causal and j >= 4 * c:
                            for h01 in range(2):
                                base = MC * h01 + off
                                nc.gpsimd.tensor_mul(
                                    probs[:, base : base + P],
                                    probs[:, base : base + P],
                                    dmask_sb,
                                )
                        probs_tiles[j] = (probs, off)

                    def attnv_step(j):
                        probs, off = probs_tiles[j]
                        for h01 in range(2):
                            h = 2 * p + h01
                            nc.tensor.matmul(
                                oaug[h01][0 : DK + 1, off:MC],
                                vaug_sb[j][:, h, :],
                                probs[:, MC * h01 + off : MC * (h01 + 1)],
                                start=(j == 0),
                                stop=(j == J - 1),
                            )

                    # software pipeline: scores two steps ahead of attnV
                    for j in range(J):
                        scores_step(j)
                        if j >= 2:
                            attnv_step(j - 2)
                    attnv_step(J - 2)
                    attnv_step(J - 1)

                    # evict o_aug to SBUF right away (frees the PSUM slot for
                    # the next group), then normalize from the SBUF copy.
                    # The reciprocal of the [1, MC] denominator row would be a
                    # single-lane DVE op (~3.3us); instead bounce it through
                    # DRAM, re-read reshaped as [128, 4] (4 elems/lane), take
                    # the reciprocal there (~0.1us), bounce back, and re-read
                    # broadcast across 64 partitions.
                    osb, bcs = [], []
                    for h01 in range(2):
                        o = rcpool.tile([DK + 1, MC], f32, tag="osb", name="osb_t")
                        nc.vector.tensor_copy(o, oaug[h01][0 : DK + 1, :])
                        osb.append(o)
                    for h01 in range(2):
                        den_d = rcdram.tile([1, MC], f32, tag="den_d", name="den_d_t")
                        nc.sync.dma_start(out=den_d, in_=osb[h01][DK : DK + 1, :])
                        den_q = rcpool.tile([P, MC // P], f32, tag="den_q", name="den_q_t")
                        nc.sync.dma_start(
                            out=den_q,
                            in_=bass.AP(
                                tensor=den_d.tensor,
                                offset=den_d.offset,
                                ap=[[MC // P, P], [1, MC // P]],
                            ),
                        )
                        rcq = rcpool.tile([P, MC // P], f32, tag="rcq", name="rcq_t")
                        nc.vector.reciprocal(rcq, den_q)
                        rcd = rcdram.tile([1, MC], f32, tag="rcd", name="rcd_t")
                        nc.sync.dma_start(
                            out=bass.AP(
                                tensor=rcd.tensor,
                                offset=rcd.offset,
                                ap=[[MC // P, P], [1, MC // P]],
                            ),
                            in_=rcq,
                        )
                        bc = rcpool.tile([64, MC], f32, tag="bc", name="bc_t")
                        nc.sync.dma_start(
                            out=bc,
                            in_=bass.AP(
                                tensor=rcd.tensor,
                                offset=rcd.offset,
                                ap=[[0, 64]] + [list(a) for a in rcd.ap[1:]],
                            ),
                        )
                        bcs.append(bc)
                    for h01 in range(2):
                        nc.vector.tensor_mul(
                            oT_sb[p][64 * h01 : 64 * (h01 + 1), MC * c : MC * (c + 1)],
                            osb[h01][0:DK, :],
                            bcs[h01],
                        )

        # ---- stage 3: output projection ----
        with tc.tile_pool(name="op_ps", bufs=4, space="PSUM") as opps:
            for t in range(NT):
                op = [
                    opps.tile([P, MC], f32, tag="op", name="op_t")
                    for _ in range(E // MC)
                ]
                for p in range(PAIRS):
                    for ec in range(E // MC):
                        nc.tensor.matmul(
                            op[ec],
                            oT_sb[p][:, P * t : P * (t + 1)],
                            wo_sb[:, p, MC * ec : MC * (ec + 1)],
                            start=(p == 0),
                            stop=(p == PAIRS - 1),
                        )
                for ec in range(E // MC):
                    ost = ostpool.tile([P, MC], f32, tag="ost", name="ost_t")
                    nc.vector.tensor_copy(ost, op[ec])
                    (nc.sync if ec == 0 else nc.scalar).dma_start(
                        out=out[P * t : P * (t + 1), MC * ec : MC * (ec + 1)],
                        in_=ost,
                    )

    nc.compile()
    return nc


def _host_inputs(key, value, query, Wk, Wq, Wv, Wo, bq, bk, bv, bias_qk, bias_v):
    """Per-core input maps (host-side shard/transpose/cast — not timed)."""
    tri = np.triu(np.ones((P, P), np.float32)).astype(BF16)  # allowed: n<=m
    in_maps = []
    xT = {}
    for b in range(B):
        xT[("q", b)] = np.ascontiguousarray(query[b].T).astype(BF16)
        xT[("k", b)] = np.ascontiguousarray(key[b].T).astype(BF16)
        xT[("v", b)] = np.ascontiguousarray(value[b].T).astype(BF16)
    for c in range(NCORES):
        b, g = divmod(c, G)
        sl = slice(DKH * g, DKH * (g + 1))
        wkqv = np.concatenate(
            [Wk[sl].T, Wq[sl].T, Wv[sl].T], axis=1
        )  # [E, 3*DKH], column blocks K|Q|V
        m = {
            "xqT": xT[("q", b)],
            "xkT": xT[("k", b)],
            "xvT": xT[("v", b)],
            "wkqvT": np.ascontiguousarray(wkqv).astype(BF16),
            "woT": np.ascontiguousarray(Wo[:, sl].T).astype(BF16),
            "dmask": tri,
        }
        if bias_qk:
            m["bq"] = np.ascontiguousarray(bq[sl].astype(np.float32).reshape(DKH, 1))
            m["bk"] = np.ascontiguousarray(bk[sl].astype(np.float32).reshape(DKH, 1))
        if bias_v:
            m["bv"] = np.ascontiguousarray(bv[sl].astype(np.float32).reshape(DKH, 1))
        in_maps.append(m)
    return in_maps


def _numpy_fallback(key, value, query, mask, Wk, bk, Wq, bq, Wv, bv, Wo, bo):
    """Exact reference semantics in numpy (general-mask fallback)."""
    def proj(x, W, b):
        return x @ W.T + b

    k = proj(key, Wk, bk).reshape(B, S, H, DK).transpose(0, 2, 1, 3)
    q = proj(query, Wq, bq).reshape(B, S, H, DK).transpose(0, 2, 1, 3)
    v = proj(value, Wv, bv).reshape(B, S, H, DK).transpose(0, 2, 1, 3)
    scores = np.einsum("bhmd,bhnd->bhmn", q, k).astype(np.float32)
    scores = np.where(mask, scores, np.float32(-1e10)) * np.float32(SCALE)
    scores -= scores.max(axis=3, keepdims=True)
    e = np.exp(scores)
    attn = e / e.sum(axis=3, keepdims=True)
    o = np.einsum("bhmn,bhnv->bhmv", attn, v)
    o = o.transpose(0, 2, 1, 3).reshape(B, S, E)
    return (o @ Wo.T + bo).astype(np.float32)


_program_cache = {}


def kernel(key, value, query, mask, Wk, bk, Wq, bq, Wv, bv, Wo, bo):
    key = np.asarray(key, np.float32)
    value = np.asarray(value, np.float32)
    query = np.asarray(query, np.float32)
    mask = np.asarray(mask)
    Wk, bk = np.asarray(Wk, np.float32), np.asarray(bk, np.float32)
    Wq, bq = np.asarray(Wq, np.float32), np.asarray(bq, np.float32)
    Wv, bv = np.asarray(Wv, np.float32), np.asarray(bv, np.float32)
    Wo, bo = np.asarray(Wo, np.float32), np.asarray(bo, np.float32)

    m2 = mask.reshape(B, S, S) if mask.size == B * S * S else None
    causal = m2 is not None and all(
        np.array_equal(m2[b], np.tril(np.ones((S, S), bool))) for b in range(B)
    )
    allones = m2 is not None and bool(mask.all())
    if not causal and not allones:
        return _numpy_fallback(key, value, query, mask, Wk, bk, Wq, bq, Wv, bv, Wo, bo)

    if causal:
        chunk_ntiles = tuple(4 * (c + 1) for c in range(NMC))
    else:
        chunk_ntiles = tuple(NT for _ in range(NMC))

    bias_qk = bool(np.any(bq) or np.any(bk))
    bias_v = bool(np.any(bv))

    pkey = (chunk_ntiles, causal, bias_qk, bias_v)
    if pkey not in _program_cache:
        _program_cache[pkey] = _build_program(chunk_ntiles, causal, bias_qk, bias_v)
    nc = _program_cache[pkey]

    from concourse.bass_utils import run_bass_kernel_spmd

    in_maps = _host_inputs(key, value, query, Wk, Wq, Wv, Wo, bq, bk, bv, bias_qk, bias_v)
    res = run_bass_kernel_spmd(nc, in_maps, core_ids=list(range(NCORES)))

    outp = np.zeros((B, S, E), np.float32)
    for c in range(NCORES):
        outp[c // G] += res.results[c]["out"]
    outp += bo.astype(np.float32)
    return outp



# revision 5
# speedup vs baseline: 1.0253x; 1.0253x over previous
"""Trainium2 Bass kernel: 16-head MHA (B=2, S=2048, E=1024) on 8 NeuronCores.

Sharding: core c = (batch b = c // 4, head-group g = c % 4); each core runs
4 heads of one batch (data parallel on B x tensor parallel on heads).  The
output projection is row-sharded: each core produces a partial [S, E] f32
output; the host sums the 4 head-group partials per batch and adds bo.

Device schedule: a single chunk-pipelined loop over the four 512-column
m-chunks.  Per chunk c the PE stream is
    proj(c):   k,q chains (weight-stationary, 8 e-tile PSUM chains)
    v(c):      v_aug computed DIRECTLY in [n, dv] layout (x-tile stationary,
               wv moving) -- no PE transposes; interleaved with
    outproj(c-1): per (m-tile, e-half) chains over both pairs
    attn(c):   scoresT (kT stationary, K=64 row-group packed), exp on ACT,
               0/1 triangle multiply on diagonal tiles (gpsimd), attnV
               (v_aug stationary) software-pipelined two steps behind
so the ACT-bound attention of chunk c overlaps the PE-only projection of
chunk c+1 and the output projection of chunk c-1.  The softmax denominator
(from the ones-column of v_aug) takes a single DRAM bounce: written [1,MC],
re-read with a step-0 partition AP as a [64,MC] broadcast, reciprocal via
the fast custom-DVE approx, then one DVE multiply into oT.  Input DMAs are
issued e-tile-granular across the sync/scalar/vector queues in consumption
order so the k-projection starts ~2us in.
"""

import numpy as np
import ml_dtypes

B, S, E = 2, 2048, 1024
H, DK = 16, 64
NCORES = 8
G = 4                 # head-groups (tensor parallel degree)
NH = H // G           # heads per core = 4
DKH = NH * DK         # 256 head dims per core
P = 128
MC = 512              # m-chunk (psum bank width in f32)
NMC = S // MC         # 4 m-chunks
NT = S // P           # 16 n-tiles (and m-tiles)
ET = E // P           # 8 e-tiles
PAIRS = NH // 2       # 2 head pairs per core
BF16 = ml_dtypes.bfloat16
SCALE = float(1.0 / np.sqrt(np.float32(DK)))


def _build_program(chunk_ntiles, causal, bias_qk, bias_v):
    """Build the (SPMD, shared across all 8 cores) Bass program.

    chunk_ntiles[c] = number of 128-wide n-tiles to process for m-chunk c.
    causal: apply diagonal-tile masking (memset + tri multiply).
    """
    from contextlib import ExitStack

    import concourse.bass as bass
    import concourse.tile as tile
    from concourse import bacc, mybir

    f32 = mybir.dt.float32
    bf16 = mybir.dt.bfloat16
    Exp = mybir.ActivationFunctionType.Exp

    nc = bacc.Bacc(
        "TRN2",
        target_bir_lowering=False,
        debug=False,
        enable_asserts=False,
        num_devices=NCORES,
    )

    # ---- DRAM I/O ----
    xqT = nc.dram_tensor("xqT", [E, S], bf16, kind="ExternalInput").ap()
    xkT = nc.dram_tensor("xkT", [E, S], bf16, kind="ExternalInput").ap()
    xvT = nc.dram_tensor("xvT", [E, S], bf16, kind="ExternalInput").ap()
    wkqvT = nc.dram_tensor("wkqvT", [E, 3 * DKH], bf16, kind="ExternalInput").ap()
    woT = nc.dram_tensor("woT", [DKH, E], bf16, kind="ExternalInput").ap()
    dmask = nc.dram_tensor("dmask", [P, P], bf16, kind="ExternalInput").ap()
    if bias_qk:
        bqd = nc.dram_tensor("bq", [DKH, 1], f32, kind="ExternalInput").ap()
        bkd = nc.dram_tensor("bk", [DKH, 1], f32, kind="ExternalInput").ap()
    if bias_v:
        bvd = nc.dram_tensor("bv", [1, DKH], f32, kind="ExternalInput").ap()
    out = nc.dram_tensor("out", [S, E], f32, kind="ExternalOutput").ap()

    with tile.TileContext(nc) as tc, ExitStack() as ctx:
        const = ctx.enter_context(tc.tile_pool(name="const", bufs=1))
        xpool = ctx.enter_context(tc.tile_pool(name="xpool", bufs=1))
        wpool = ctx.enter_context(tc.tile_pool(name="wpool", bufs=1))
        qkpool = ctx.enter_context(tc.tile_pool(name="qkpool", bufs=1))
        vpool = ctx.enter_context(tc.tile_pool(name="vpool", bufs=1))
        prpool = ctx.enter_context(tc.tile_pool(name="prpool", bufs=8))
        rcpool = ctx.enter_context(tc.tile_pool(name="rcpool", bufs=2))
        otpool = ctx.enter_context(tc.tile_pool(name="otpool", bufs=1))
        ostpool = ctx.enter_context(tc.tile_pool(name="ostpool", bufs=4))
        # PSUM: "pj" (proj + outproj chains) 2 banks, "sc" 4 banks,
        # "oaug" 2 banks -- exactly the 8 banks.
        pjps = ctx.enter_context(tc.tile_pool(name="pj_ps", bufs=2, space="PSUM"))
        scps = ctx.enter_context(tc.tile_pool(name="sc_ps", bufs=2, space="PSUM"))
        oaps = ctx.enter_context(tc.tile_pool(name="oa_ps", bufs=2, space="PSUM"))
        rcdram = ctx.enter_context(tc.tile_pool(name="rc_dram", bufs=4, space="DRAM"))

        # ---- persistent SBUF tiles ----
        wkqv_sb = wpool.tile([P, ET, 3 * DKH], bf16, tag="wkqv")
        wo_sb = wpool.tile([P, PAIRS, E], bf16, tag="wo")
        xk_t = xpool.tile([P, ET, S], bf16, tag="xk")
        xq_t = xpool.tile([P, ET, S], bf16, tag="xq")
        xv_t = xpool.tile([P, ET, S], bf16, tag="xv")
        dmask_sb = const.tile([P, P], bf16, tag="dmask")

        # ---- input DMA issue: e-tile granular, consumption order ----
        # k path first (first PE chains), then q, then v.  Three HWDGE
        # queues (sync/scalar/vector) run transfers concurrently; issue
        # cost is ~0.6us each on the issuing engine's sequencer.
        def wk_dma(i):
            return (wkqv_sb[:, i, :], wkqvT[P * i : P * (i + 1), :])

        def x_dma(x_sb, xT, i):
            src = xT.rearrange("(t p) s -> p t s", p=P)[:, i, :]
            return (x_sb[:, i, :], src)

        sync_q = [wk_dma(0)] + [x_dma(xk_t, xkT, i) for i in (0, 2, 4, 6)] + [
            x_dma(xq_t, xqT, i) for i in (1, 3, 5, 7)
        ]
        scalar_q = []
        for i in range(1, ET):
            scalar_q.append(wk_dma(i))
            if i < 5:
                scalar_q.append(x_dma(xk_t, xkT, 2 * i - 1))
            else:
                scalar_q.append(x_dma(xq_t, xqT, 2 * (i - 5)))
        scalar_q.append(x_dma(xq_t, xqT, 6))
        scalar_q += [
            (wo_sb[:, p, :], woT[P * p : P * (p + 1), :]) for p in range(PAIRS)
        ]
        scalar_q.append((dmask_sb, dmask))
        # xv through gpsimd's SWDGE: a third concurrent DMA stream, and the
        # Pool engine is otherwise idle until the first diagonal-mask multiply
        gpsimd_q = [x_dma(xv_t, xvT, i) for i in range(ET)]
        for eng, q in ((nc.sync, sync_q), (nc.scalar, scalar_q), (nc.gpsimd, gpsimd_q)):
            for dst, src in q:
                eng.dma_start(out=dst, in_=src)

        if bias_qk:
            bq_sb = const.tile([P, PAIRS], f32, tag="bq")
            nc.sync.dma_start(out=bq_sb, in_=bqd.rearrange("(t p) o -> p (t o)", p=P))
            bk_sb = const.tile([P, PAIRS], f32, tag="bk")
            nc.sync.dma_start(out=bk_sb, in_=bkd.rearrange("(t p) o -> p (t o)", p=P))
        if bias_v:
            # bv along the free dim of the direct v_aug layout: broadcast
            # [1, DKH] across all partitions via a step-0 partition AP.
            bv_sb = const.tile([P, DKH], f32, tag="bv")
            nc.sync.dma_start(
                out=bv_sb,
                in_=bass.AP(
                    tensor=bvd.tensor,
                    offset=bvd.offset,
                    ap=[[0, P]] + [list(a) for a in bvd.ap[1:]],
                ),
            )

        # persistent activation tiles
        qT_sb = [qkpool.tile([P, S], bf16, tag=f"qT{p}", name=f"qT_sb{p}") for p in range(PAIRS)]
        kT_sb = [qkpool.tile([P, S], bf16, tag=f"kT{p}", name=f"kT_sb{p}") for p in range(PAIRS)]
        vaug_sb = [vpool.tile([P, NH, DK + 1], bf16, tag=f"va{j}", name=f"vaug_sb{j}") for j in range(NT)]
        oT_sb = [otpool.tile([P, S], bf16, tag=f"oT{p}", name=f"oT_sb{p}") for p in range(PAIRS)]

        wv0 = 2 * DKH

        def kq_chains(c):
            for dst, w0, x_t, bias in (
                (kT_sb, 0, xk_t, bk_sb if bias_qk else None),
                (qT_sb, DKH, xq_t, bq_sb if bias_qk else None),
            ):
                for p in range(PAIRS):
                    ps = pjps.tile([P, MC], f32, tag="pj", name="ps_kq")
                    for i in range(ET):
                        nc.tensor.matmul(
                            ps,
                            wkqv_sb[:, i, w0 + P * p : w0 + P * (p + 1)],
                            x_t[:, i, MC * c : MC * (c + 1)],
                            start=(i == 0),
                            stop=(i == ET - 1),
                        )
                    dslice = dst[p][:, MC * c : MC * (c + 1)]
                    if bias is not None:
                        nc.vector.tensor_scalar_add(dslice, ps, bias[:, p : p + 1])
                    else:
                        nc.vector.tensor_copy(dslice, ps)

        def v_chain(j):
            # v_aug[n-tile j] = xv-tile stationary, wv moving: [128 n, 256 dv]
            ps = pjps.tile([P, MC], f32, tag="pj", name="ps_v")
            pv = ps[:, 0:DKH]
            for i in range(ET):
                nc.tensor.matmul(
                    pv,
                    xv_t[:, i, P * j : P * (j + 1)],
                    wkqv_sb[:, i, wv0 : wv0 + DKH],
                    start=(i == 0),
                    stop=(i == ET - 1),
                )
            if bias_v:
                nc.vector.tensor_add(pv, pv, bv_sb)
            pt3 = pv.rearrange("n (h d) -> n h d", h=NH)
            nc.vector.tensor_copy(vaug_sb[j][:, :, 0:DK], pt3)
            nc.gpsimd.memset(vaug_sb[j][:, :, DK : DK + 1], 1.0)

        def op_chain(t, ec, evict_eng, dma_eng):
            op = pjps.tile([P, MC], f32, tag="pj", name="op_t")
            for p in range(PAIRS):
                nc.tensor.matmul(
                    op,
                    oT_sb[p][:, P * t : P * (t + 1)],
                    wo_sb[:, p, MC * ec : MC * (ec + 1)],
                    start=(p == 0),
                    stop=(p == PAIRS - 1),
                )
            ost = ostpool.tile([P, MC], f32, tag="ost", name="ost_t")
            if evict_eng is nc.scalar:
                nc.scalar.copy(ost, op)
            else:
                evict_eng.tensor_copy(ost, op)
            dma_eng.dma_start(
                out=out[P * t : P * (t + 1), MC * ec : MC * (ec + 1)], in_=ost
            )

        def attn_group(c, p):
            J = chunk_ntiles[c]
            oaug = [
                oaps.tile([P, MC], f32, tag="oaug", name=f"oaug{h01}")
                for h01 in range(2)
            ]
            probs_tiles = [None] * J

            def scores_step(j):
                # columns left of `off` in this m-chunk are fully masked
                # for n-tile j: never compute/exp/consume them
                off = P * (j - 4 * c) if (causal and j >= 4 * c) else 0
                sc = scps.tile([P, 2 * MC], f32, tag="sc", name="sc_ps_t")
                for h01 in range(2):
                    nc.tensor.matmul(
                        sc[:, MC * h01 + off : MC * (h01 + 1)],
                        kT_sb[p][64 * h01 : 64 * (h01 + 1), P * j : P * (j + 1)],
                        qT_sb[p][64 * h01 : 64 * (h01 + 1), MC * c + off : MC * (c + 1)],
                        start=True,
                        stop=True,
                    )
                probs = prpool.tile([P, 2 * MC], bf16, tag="probs", name="probs_t")
                sc3 = sc.rearrange("p (u m) -> p u m", u=2)
                pr3 = probs.rearrange("p (u m) -> p u m", u=2)
                nc.scalar.activation(
                    pr3[:, :, off:MC], sc3[:, :, off:MC], Exp, bias=0.0, scale=SCALE
                )
                if causal and j >= 4 * c:
                    for h01 in range(2):
                        base = MC * h01 + off
                        nc.gpsimd.tensor_mul(
                            probs[:, base : base + P],
                            probs[:, base : base + P],
                            dmask_sb,
                        )
                probs_tiles[j] = (probs, off)

            def attnv_step(j):
                probs, off = probs_tiles[j]
                for h01 in range(2):
                    h = 2 * p + h01
                    nc.tensor.matmul(
                        oaug[h01][0 : DK + 1, off:MC],
                        vaug_sb[j][:, h, :],
                        probs[:, MC * h01 + off : MC * (h01 + 1)],
                        start=(j == 0),
                        stop=(j == J - 1),
                    )

            # software pipeline: scores two steps ahead of attnV
            for j in range(J):
                scores_step(j)
                if j >= 2:
                    attnv_step(j - 2)
            attnv_step(J - 2)
            attnv_step(J - 1)

            # epilogue: evict o_aug to SBUF (frees PSUM), bounce the
            # denominator row once through DRAM to re-read it broadcast
            # across 64 partitions, fast-reciprocal, multiply into oT.
            osb = []
            for h01 in range(2):
                o = rcpool.tile([DK + 1, MC], f32, tag="osb", bufs=4, name="osb_t")
                nc.vector.tensor_copy(o, oaug[h01][0 : DK + 1, :])
                osb.append(o)
            for h01 in range(2):
                den_d = rcdram.tile([1, MC], f32, tag="den_d", name="den_d_t")
                nc.sync.dma_start(out=den_d, in_=osb[h01][DK : DK + 1, :])
                bcden = rcpool.tile([64, MC], f32, tag="bc", bufs=2, name="bc_t")
                nc.sync.dma_start(
                    out=bcden,
                    in_=bass.AP(
                        tensor=den_d.tensor,
                        offset=den_d.offset,
                        ap=[[0, 64]] + [list(a) for a in den_d.ap[1:]],
                    ),
                )
                rc = rcpool.tile([64, MC], f32, tag="rc", bufs=2, name="rc_t")
                nc.vector.reciprocal_approx_fast(out=rc, in_=bcden)
                nc.vector.tensor_mul(
                    oT_sb[p][64 * h01 : 64 * (h01 + 1), MC * c : MC * (c + 1)],
                    osb[h01][0:DK, :],
                    rc,
                )

        # ---- the chunk-pipelined schedule ----
        for c in range(NMC):
            kq_chains(c)
            # v tiles needed by attn(c): causal -> this chunk's 4 tiles;
            # non-causal -> everything up front (attn(0) consumes all).
            if causal:
                vjs = list(range(4 * c, 4 * c + 4))
            else:
                vjs = list(range(NT)) if c == 0 else []
            ops = []
            if c > 0:
                ops = [
                    (t, ec)
                    for t in range(4 * (c - 1), 4 * c)
                    for ec in range(E // MC)
                ]
            # weave: v chain then up to 2 outproj chains, repeating
            oi = 0
            for vi, j in enumerate(vjs):
                v_chain(j)
                take = 2 if vi < len(vjs) - 1 else len(ops) - oi
                for _ in range(min(take, max(0, len(ops) - oi))):
                    t, ec = ops[oi]
                    op_chain(t, ec, nc.vector, nc.sync)
                    oi += 1
            while oi < len(ops):
                t, ec = ops[oi]
                op_chain(t, ec, nc.vector, nc.sync)
                oi += 1
            for p in range(PAIRS):
                attn_group(c, p)

        # tail: outproj of the last chunk (ACT is idle now -> scalar evicts)
        n = 0
        for t in range(4 * (NMC - 1), 4 * NMC):
            for ec in range(E // MC):
                op_chain(
                    t,
                    ec,
                    nc.scalar if n % 2 == 0 else nc.vector,
                    nc.sync if n % 2 == 0 else nc.scalar,
                )
                n += 1

    nc.compile()
    return nc


def _host_inputs(key, value, query, Wk, Wq, Wv, Wo, bq, bk, bv, bias_qk, bias_v):
    """Per-core input maps (host-side shard/transpose/cast — not timed)."""
    tri = np.triu(np.ones((P, P), np.float32)).astype(BF16)  # allowed: n<=m
    in_maps = []
    xT = {}
    for b in range(B):
        xT[("q", b)] = np.ascontiguousarray(query[b].T).astype(BF16)
        xT[("k", b)] = np.ascontiguousarray(key[b].T).astype(BF16)
        xT[("v", b)] = np.ascontiguousarray(value[b].T).astype(BF16)
    for c in range(NCORES):
        b, g = divmod(c, G)
        sl = slice(DKH * g, DKH * (g + 1))
        wkqv = np.concatenate(
            [Wk[sl].T, Wq[sl].T, Wv[sl].T], axis=1
        )  # [E, 3*DKH], column blocks K|Q|V
        m = {
            "xqT": xT[("q", b)],
            "xkT": xT[("k", b)],
            "xvT": xT[("v", b)],
            "wkqvT": np.ascontiguousarray(wkqv).astype(BF16),
            "woT": np.ascontiguousarray(Wo[:, sl].T).astype(BF16),
            "dmask": tri,
        }
        if bias_qk:
            m["bq"] = np.ascontiguousarray(bq[sl].astype(np.float32).reshape(DKH, 1))
            m["bk"] = np.ascontiguousarray(bk[sl].astype(np.float32).reshape(DKH, 1))
        if bias_v:
            m["bv"] = np.ascontiguousarray(bv[sl].astype(np.float32).reshape(1, DKH))
        in_maps.append(m)
    return in_maps


def _numpy_fallback(key, value, query, mask, Wk, bk, Wq, bq, Wv, bv, Wo, bo):
    """Exact reference semantics in numpy (general-mask fallback)."""
    def proj(x, W, b):
        return x @ W.T + b

    k = proj(key, Wk, bk).reshape(B, S, H, DK).transpose(0, 2, 1, 3)
    q = proj(query, Wq, bq).reshape(B, S, H, DK).transpose(0, 2, 1, 3)
    v = proj(value, Wv, bv).reshape(B, S, H, DK).transpose(0, 2, 1, 3)
    scores = np.einsum("bhmd,bhnd->bhmn", q, k).astype(np.float32)
    scores = np.where(mask, scores, np.float32(-1e10)) * np.float32(SCALE)
    scores -= scores.max(axis=3, keepdims=True)
    e = np.exp(scores)
    attn = e / e.sum(axis=3, keepdims=True)
    o = np.einsum("bhmn,bhnv->bhmv", attn, v)
    o = o.transpose(0, 2, 1, 3).reshape(B, S, E)
    return (o @ Wo.T + bo).astype(np.float32)


_program_cache = {}


def kernel(key, value, query, mask, Wk, bk, Wq, bq, Wv, bv, Wo, bo):
    key = np.asarray(key, np.float32)
    value = np.asarray(value, np.float32)
    query = np.asarray(query, np.float32)
    mask = np.asarray(mask)
    Wk, bk = np.asarray(Wk, np.float32), np.asarray(bk, np.float32)
    Wq, bq = np.asarray(Wq, np.float32), np.asarray(bq, np.float32)
    Wv, bv = np.asarray(Wv, np.float32), np.asarray(bv, np.float32)
    Wo, bo = np.asarray(Wo, np.float32), np.asarray(bo, np.float32)

    m2 = mask.reshape(B, S, S) if mask.size == B * S * S else None
    causal = m2 is not None and all(
        np.array_equal(m2[b], np.tril(np.ones((S, S), bool))) for b in range(B)
    )
    allones = m2 is not None and bool(mask.all())
    if not causal and not allones:
        return _numpy_fallback(key, value, query, mask, Wk, bk, Wq, bq, Wv, bv, Wo, bo)

    if causal:
        chunk_ntiles = tuple(4 * (c + 1) for c in range(NMC))
    else:
        chunk_ntiles = tuple(NT for _ in range(NMC))

    bias_qk = bool(np.any(bq) or np.any(bk))
    bias_v = bool(np.any(bv))

    pkey = (chunk_ntiles, causal, bias_qk, bias_v)
    if pkey not in _program_cache:
        _program_cache[pkey] = _build_program(chunk_ntiles, causal, bias_qk, bias_v)
    nc = _program_cache[pkey]

    from concourse.bass_utils import run_bass_kernel_spmd

    in_maps = _host_inputs(key, value, query, Wk, Wq, Wv, Wo, bq, bk, bv, bias_qk, bias_v)
    res = run_bass_kernel_spmd(nc, in_maps, core_ids=list(range(NCORES)))

    outp = np.zeros((B, S, E), np.float32)
    for c in range(NCORES):
        outp[c // G] += res.results[c]["out"]
    outp += bo.astype(np.float32)
    return outp


# revision 11
# speedup vs baseline: 1.1258x; 1.0980x over previous
"""Trainium2 Bass kernel: 16-head MHA (B=2, S=2048, E=1024) on 8 NeuronCores.

Sharding: core c = (batch b = c // 4, head-group g = c % 4); each core runs
4 heads of one batch (data parallel on B x tensor parallel on heads).  The
output projection is row-sharded: each core produces a partial [S, E] f32
output; the host sums the 4 head-group partials per batch and adds bo.

Device schedule: a single chunk-pipelined loop over the four 512-column
m-chunks.  Per chunk c the PE stream is
    proj(c):   k,q chains (weight-stationary, 8 e-tile PSUM chains)
    v(c):      v_aug computed DIRECTLY in [n, dv] layout (x-tile stationary,
               wv moving) -- no PE transposes; interleaved with
    outproj(c-1): per (m-tile, e-half) chains over both pairs
    attn(c):   scoresT (kT stationary, K=64 row-group packed), exp on ACT,
               0/1 triangle multiply on diagonal tiles (gpsimd), attnV
               (v_aug stationary) software-pipelined two steps behind
so the ACT-bound attention of chunk c overlaps the PE-only projection of
chunk c+1 and the output projection of chunk c-1.  The softmax denominator
(from the ones-column of v_aug) takes a single DRAM bounce: written [1,MC],
re-read with a step-0 partition AP as a [64,MC] broadcast, reciprocal via
the fast custom-DVE approx, then one DVE multiply into oT.  Input DMAs are
issued e-tile-granular across the sync/scalar/vector queues in consumption
order so the k-projection starts ~2us in.
"""

import numpy as np
import ml_dtypes

B, S, E = 2, 2048, 1024
H, DK = 16, 64
NCORES = 8
G = 4                 # head-groups (tensor parallel degree)
NH = H // G           # heads per core = 4
DKH = NH * DK         # 256 head dims per core
P = 128
MC = 512              # m-chunk (psum bank width in f32)
NMC = S // MC         # 4 m-chunks
NT = S // P           # 16 n-tiles (and m-tiles)
ET = E // P           # 8 e-tiles
PAIRS = NH // 2       # 2 head pairs per core
BF16 = ml_dtypes.bfloat16
SCALE = float(1.0 / np.sqrt(np.float32(DK)))


def _build_program(chunk_ntiles, causal, bias_qk, bias_v):
    """Build the (SPMD, shared across all 8 cores) Bass program.

    chunk_ntiles[c] = number of 128-wide n-tiles to process for m-chunk c.
    causal: apply diagonal-tile masking (memset + tri multiply).
    """
    from contextlib import ExitStack

    import concourse.bass as bass
    import concourse.tile as tile
    from concourse import bacc, mybir

    f32 = mybir.dt.float32
    bf16 = mybir.dt.bfloat16
    Exp = mybir.ActivationFunctionType.Exp

    nc = bacc.Bacc(
        "TRN2",
        target_bir_lowering=False,
        debug=False,
        enable_asserts=False,
        num_devices=NCORES,
    )

    # ---- DRAM I/O ----
    xqT = nc.dram_tensor("xqT", [E, S], bf16, kind="ExternalInput").ap()
    xkT = nc.dram_tensor("xkT", [E, S], bf16, kind="ExternalInput").ap()
    xvT = nc.dram_tensor("xvT", [E, S], bf16, kind="ExternalInput").ap()
    wkqvT = nc.dram_tensor("wkqvT", [E, 3 * DKH], bf16, kind="ExternalInput").ap()
    woT = nc.dram_tensor("woT", [DKH, E], bf16, kind="ExternalInput").ap()
    dmask = nc.dram_tensor("dmask", [P, P], bf16, kind="ExternalInput").ap()
    if bias_qk:
        bqd = nc.dram_tensor("bq", [DKH, 1], f32, kind="ExternalInput").ap()
        bkd = nc.dram_tensor("bk", [DKH, 1], f32, kind="ExternalInput").ap()
    if bias_v:
        bvd = nc.dram_tensor("bv", [1, DKH], f32, kind="ExternalInput").ap()
    out = nc.dram_tensor("out", [S, E], f32, kind="ExternalOutput").ap()

    with tile.TileContext(nc) as tc, ExitStack() as ctx:
        const = ctx.enter_context(tc.tile_pool(name="const", bufs=1))
        xpool = ctx.enter_context(tc.tile_pool(name="xpool", bufs=1))
        wpool = ctx.enter_context(tc.tile_pool(name="wpool", bufs=1))
        qkpool = ctx.enter_context(tc.tile_pool(name="qkpool", bufs=1))
        vpool = ctx.enter_context(tc.tile_pool(name="vpool", bufs=1))
        prpool = ctx.enter_context(tc.tile_pool(name="prpool", bufs=8))
        rcpool = ctx.enter_context(tc.tile_pool(name="rcpool", bufs=2))
        otpool = ctx.enter_context(tc.tile_pool(name="otpool", bufs=1))
        ostpool = ctx.enter_context(tc.tile_pool(name="ostpool", bufs=4))
        # PSUM: "pj" (proj + outproj chains) 2 banks, "sc" 4 banks,
        # "oaug" 2 banks -- exactly the 8 banks.
        pjps = ctx.enter_context(tc.tile_pool(name="pj_ps", bufs=2, space="PSUM"))
        scps = ctx.enter_context(tc.tile_pool(name="sc_ps", bufs=2, space="PSUM"))
        oaps = ctx.enter_context(tc.tile_pool(name="oa_ps", bufs=2, space="PSUM"))
        rcdram = ctx.enter_context(tc.tile_pool(name="rc_dram", bufs=4, space="DRAM"))

        # ---- persistent SBUF tiles ----
        wkqv_sb = wpool.tile([P, ET, 3 * DKH], bf16, tag="wkqv")
        wo_sb = wpool.tile([P, PAIRS, E], bf16, tag="wo")
        xk_t = xpool.tile([P, ET, S], bf16, tag="xk")
        xq_t = xpool.tile([P, ET, S], bf16, tag="xq")
        xv_t = xpool.tile([P, ET, S], bf16, tag="xv")
        dmask_sb = const.tile([P, P], bf16, tag="dmask")

        # ---- input DMA issue: stripe every tensor across all three DMA
        # queues (sync/scalar HWDGE + gpsimd SWDGE) in consumption order
        # (weights -> xk -> xq -> xv).  A single queue sustains only
        # ~130GB/s, so landing a 4MB tensor fast needs all three.
        def wk_dma(i):
            return (wkqv_sb[:, i, :], wkqvT[P * i : P * (i + 1), :])

        def x_dma(x_sb, xT, i):
            src = xT.rearrange("(t p) s -> p t s", p=P)[:, i, :]
            return (x_sb[:, i, :], src)

        sync_q = [(dmask_sb, dmask)]
        gpsimd_q = []
        scalar_q = []
        queues = [sync_q, gpsimd_q, scalar_q]
        for group in (
            [wk_dma(i) for i in range(ET)],
            [x_dma(xk_t, xkT, i) for i in range(ET)],
            [x_dma(xq_t, xqT, i) for i in range(ET)],
            [x_dma(xv_t, xvT, i) for i in range(ET)],
        ):
            for i, tr in enumerate(group):
                queues[i % 3].append(tr)
        scalar_q += [
            (wo_sb[:, p, :], woT[P * p : P * (p + 1), :]) for p in range(PAIRS)
        ]
        for eng, q in ((nc.sync, sync_q), (nc.scalar, scalar_q), (nc.gpsimd, gpsimd_q)):
            for dst, src in q:
                eng.dma_start(out=dst, in_=src)

        if bias_qk:
            bq_sb = const.tile([P, PAIRS], f32, tag="bq")
            nc.sync.dma_start(out=bq_sb, in_=bqd.rearrange("(t p) o -> p (t o)", p=P))
            bk_sb = const.tile([P, PAIRS], f32, tag="bk")
            nc.sync.dma_start(out=bk_sb, in_=bkd.rearrange("(t p) o -> p (t o)", p=P))
        if bias_v:
            # bv along the free dim of the direct v_aug layout: broadcast
            # [1, DKH] across all partitions via a step-0 partition AP.
            bv_sb = const.tile([P, DKH], f32, tag="bv")
            nc.sync.dma_start(
                out=bv_sb,
                in_=bass.AP(
                    tensor=bvd.tensor,
                    offset=bvd.offset,
                    ap=[[0, P]] + [list(a) for a in bvd.ap[1:]],
                ),
            )

        # persistent activation tiles
        qT_sb = [qkpool.tile([P, S], bf16, tag=f"qT{p}", name=f"qT_sb{p}") for p in range(PAIRS)]
        kT_sb = [qkpool.tile([P, S], bf16, tag=f"kT{p}", name=f"kT_sb{p}") for p in range(PAIRS)]
        vaug_sb = [vpool.tile([P, NH, DK + 1], bf16, tag=f"va{j}", name=f"vaug_sb{j}") for j in range(NT)]
        oT_sb = [otpool.tile([P, S], bf16, tag=f"oT{p}", name=f"oT_sb{p}") for p in range(PAIRS)]

        wv0 = 2 * DKH

        def proj_chain(dst, w0, x_t, bias, p, c):
            ps = pjps.tile([P, MC], f32, tag="pj", name="ps_kq")
            for i in range(ET):
                nc.tensor.matmul(
                    ps,
                    wkqv_sb[:, i, w0 + P * p : w0 + P * (p + 1)],
                    x_t[:, i, MC * c : MC * (c + 1)],
                    start=(i == 0),
                    stop=(i == ET - 1),
                )
            dslice = dst[p][:, MC * c : MC * (c + 1)]
            if bias is not None:
                nc.vector.tensor_scalar_add(dslice, ps, bias[:, p : p + 1])
            else:
                nc.vector.tensor_copy(dslice, ps)

        def v_chain(j):
            # v_aug[n-tile j] = xv-tile stationary, wv moving: [128 n, 256 dv]
            ps = pjps.tile([P, MC], f32, tag="pj", name="ps_v")
            pv = ps[:, 0:DKH]
            for i in range(ET):
                nc.tensor.matmul(
                    pv,
                    xv_t[:, i, P * j : P * (j + 1)],
                    wkqv_sb[:, i, wv0 : wv0 + DKH],
                    start=(i == 0),
                    stop=(i == ET - 1),
                )
            if bias_v:
                nc.vector.tensor_add(pv, pv, bv_sb)
            pt3 = pv.rearrange("n (h d) -> n h d", h=NH)
            nc.vector.tensor_copy(vaug_sb[j][:, :, 0:DK], pt3)
            nc.gpsimd.memset(vaug_sb[j][:, :, DK : DK + 1], 1.0)

        def op_chain(t, ec, evict_eng, dma_eng):
            op = pjps.tile([P, MC], f32, tag="pj", name="op_t")
            for p in range(PAIRS):
                nc.tensor.matmul(
                    op,
                    oT_sb[p][:, P * t : P * (t + 1)],
                    wo_sb[:, p, MC * ec : MC * (ec + 1)],
                    start=(p == 0),
                    stop=(p == PAIRS - 1),
                )
            ost = ostpool.tile([P, MC], f32, tag="ost", name="ost_t")
            if evict_eng is nc.scalar:
                nc.scalar.copy(ost, op)
            else:
                evict_eng.tensor_copy(ost, op)
            dma_eng.dma_start(
                out=out[P * t : P * (t + 1), MC * ec : MC * (ec + 1)], in_=ost
            )

        def attn_group(c, p, fillers):
            """fillers: list of zero-arg callables emitting PE filler chains
            (v_aug builds, outproj chains); one is consumed right before
            each attnV step so the PE has work while ACT runs exp."""
            J = chunk_ntiles[c]
            oaug = [
                oaps.tile([P, MC], f32, tag="oaug", name=f"oaug{h01}")
                for h01 in range(2)
            ]
            probs_tiles = [None] * J

            def scores_step(j):
                # columns left of `off` in this m-chunk are fully masked
                # for n-tile j: never compute/exp/consume them
                off = P * (j - 4 * c) if (causal and j >= 4 * c) else 0
                sc = scps.tile([P, 2 * MC], f32, tag="sc", name="sc_ps_t")
                for h01 in range(2):
                    nc.tensor.matmul(
                        sc[:, MC * h01 + off : MC * (h01 + 1)],
                        kT_sb[p][64 * h01 : 64 * (h01 + 1), P * j : P * (j + 1)],
                        qT_sb[p][64 * h01 : 64 * (h01 + 1), MC * c + off : MC * (c + 1)],
                        start=True,
                        stop=True,
                    )
                probs = prpool.tile([P, 2 * MC], bf16, tag="probs", name="probs_t")
                sc3 = sc.rearrange("p (u m) -> p u m", u=2)
                pr3 = probs.rearrange("p (u m) -> p u m", u=2)
                nc.scalar.activation(
                    pr3[:, :, off:MC], sc3[:, :, off:MC], Exp, bias=0.0, scale=SCALE
                )
                if causal and j >= 4 * c:
                    for h01 in range(2):
                        base = MC * h01 + off
                        nc.gpsimd.tensor_mul(
                            probs[:, base : base + P],
                            probs[:, base : base + P],
                            dmask_sb,
                        )
                probs_tiles[j] = (probs, off)

            def attnv_step(j):
                probs, off = probs_tiles[j]
                for h01 in range(2):
                    h = 2 * p + h01
                    nc.tensor.matmul(
                        oaug[h01][0 : DK + 1, off:MC],
                        vaug_sb[j][:, h, :],
                        probs[:, MC * h01 + off : MC * (h01 + 1)],
                        start=(j == 0),
                        stop=(j == J - 1),
                    )

            # software pipeline: scores two steps ahead of attnV, with one
            # PE filler chain in front of each attnV step
            def take_filler():
                if fillers:
                    fillers.pop(0)()

            for j in range(J):
                scores_step(j)
                if j >= 2:
                    take_filler()
                    attnv_step(j - 2)
            take_filler()
            attnv_step(J - 2)
            take_filler()
            attnv_step(J - 1)

            # epilogue: evict o_aug to SBUF (frees PSUM); bounce the
            # denominator row once through DRAM to re-read it broadcast
            # across 64 partitions (step-0 partition AP), fast-reciprocal,
            # multiply into oT.
            osb = []
            for h01 in range(2):
                o = rcpool.tile([DK + 1, MC], f32, tag="osb", bufs=4, name="osb_t")
                nc.vector.tensor_copy(o, oaug[h01][0 : DK + 1, :])
                osb.append(o)
            for h01 in range(2):
                den_d = rcdram.tile([1, MC], f32, tag="den_d", name="den_d_t")
                nc.sync.dma_start(out=den_d, in_=osb[h01][DK : DK + 1, :])
                bcden = rcpool.tile([64, MC], f32, tag="bc", bufs=2, name="bc_t")
                nc.sync.dma_start(
                    out=bcden,
                    in_=bass.AP(
                        tensor=den_d.tensor,
                        offset=den_d.offset,
                        ap=[[0, 64]] + [list(a) for a in den_d.ap[1:]],
                    ),
                )
                rc = rcpool.tile([64, MC], f32, tag="rc", bufs=2, name="rc_t")
                nc.vector.reciprocal_approx_fast(out=rc, in_=bcden)
                nc.vector.tensor_mul(
                    oT_sb[p][64 * h01 : 64 * (h01 + 1), MC * c : MC * (c + 1)],
                    osb[h01][0:DK, :],
                    rc,
                )

        # ---- schedule ----
        # All k chains first (paced by the arriving xk stripe), then per
        # chunk: q chains + the two attention groups.  v_aug builds and the
        # previous chunk's outproj chains ride inside the attention groups
        # as PE fillers (one per attnV step), so the exp-paced attention
        # windows keep the PE fed.
        kb = bk_sb if bias_qk else None
        qb = bq_sb if bias_qk else None
        for c in range(NMC):
            for p in range(PAIRS):
                proj_chain(kT_sb, 0, xk_t, kb, p, c)
        for c in range(NMC):
            for p in range(PAIRS):
                proj_chain(qT_sb, DKH, xq_t, qb, p, c)
            if causal:
                vjs = list(range(4 * c, 4 * c + 4))
            else:
                vjs = list(range(NT)) if c == 0 else []
            fillers = [
                (lambda j=j: v_chain(j)) for j in vjs
            ]
            if c > 0:
                fillers += [
                    (lambda t=t, ec=ec: op_chain(t, ec, nc.vector, nc.sync))
                    for t in range(4 * (c - 1), 4 * c)
                    for ec in range(E // MC)
                ]
            for p in range(PAIRS):
                attn_group(c, p, fillers)
            for f in fillers:
                f()
            del fillers[:]

        # tail: outproj of the last chunk (ACT is idle now -> scalar evicts)
        n = 0
        for t in range(4 * (NMC - 1), 4 * NMC):
            for ec in range(E // MC):
                op_chain(
                    t,
                    ec,
                    nc.scalar if n % 2 == 0 else nc.vector,
                    nc.sync if n % 2 == 0 else nc.scalar,
                )
                n += 1

    nc.compile()
    return nc


def _host_inputs(key, value, query, Wk, Wq, Wv, Wo, bq, bk, bv, bias_qk, bias_v):
    """Per-core input maps (host-side shard/transpose/cast — not timed)."""
    tri = np.triu(np.ones((P, P), np.float32)).astype(BF16)  # allowed: n<=m
    in_maps = []
    xT = {}
    for b in range(B):
        xT[("q", b)] = np.ascontiguousarray(query[b].T).astype(BF16)
        xT[("k", b)] = np.ascontiguousarray(key[b].T).astype(BF16)
        xT[("v", b)] = np.ascontiguousarray(value[b].T).astype(BF16)
    for c in range(NCORES):
        b, g = divmod(c, G)
        sl = slice(DKH * g, DKH * (g + 1))
        wkqv = np.concatenate(
            [Wk[sl].T, Wq[sl].T, Wv[sl].T], axis=1
        )  # [E, 3*DKH], column blocks K|Q|V
        m = {
            "xqT": xT[("q", b)],
            "xkT": xT[("k", b)],
            "xvT": xT[("v", b)],
            "wkqvT": np.ascontiguousarray(wkqv).astype(BF16),
            "woT": np.ascontiguousarray(Wo[:, sl].T).astype(BF16),
            "dmask": tri,
        }
        if bias_qk:
            m["bq"] = np.ascontiguousarray(bq[sl].astype(np.float32).reshape(DKH, 1))
            m["bk"] = np.ascontiguousarray(bk[sl].astype(np.float32).reshape(DKH, 1))
        if bias_v:
            m["bv"] = np.ascontiguousarray(bv[sl].astype(np.float32).reshape(1, DKH))
        in_maps.append(m)
    return in_maps


def _numpy_fallback(key, value, query, mask, Wk, bk, Wq, bq, Wv, bv, Wo, bo):
    """Exact reference semantics in numpy (general-mask fallback)."""
    def proj(x, W, b):
        return x @ W.T + b

    k = proj(key, Wk, bk).reshape(B, S, H, DK).transpose(0, 2, 1, 3)
    q = proj(query, Wq, bq).reshape(B, S, H, DK).transpose(0, 2, 1, 3)
    v = proj(value, Wv, bv).reshape(B, S, H, DK).transpose(0, 2, 1, 3)
    scores = np.einsum("bhmd,bhnd->bhmn", q, k).astype(np.float32)
    scores = np.where(mask, scores, np.float32(-1e10)) * np.float32(SCALE)
    scores -= scores.max(axis=3, keepdims=True)
    e = np.exp(scores)
    attn = e / e.sum(axis=3, keepdims=True)
    o = np.einsum("bhmn,bhnv->bhmv", attn, v)
    o = o.transpose(0, 2, 1, 3).reshape(B, S, E)
    return (o @ Wo.T + bo).astype(np.float32)


_program_cache = {}


def kernel(key, value, query, mask, Wk, bk, Wq, bq, Wv, bv, Wo, bo):
    key = np.asarray(key, np.float32)
    value = np.asarray(value, np.float32)
    query = np.asarray(query, np.float32)
    mask = np.asarray(mask)
    Wk, bk = np.asarray(Wk, np.float32), np.asarray(bk, np.float32)
    Wq, bq = np.asarray(Wq, np.float32), np.asarray(bq, np.float32)
    Wv, bv = np.asarray(Wv, np.float32), np.asarray(bv, np.float32)
    Wo, bo = np.asarray(Wo, np.float32), np.asarray(bo, np.float32)

    m2 = mask.reshape(B, S, S) if mask.size == B * S * S else None
    causal = m2 is not None and all(
        np.array_equal(m2[b], np.tril(np.ones((S, S), bool))) for b in range(B)
    )
    allones = m2 is not None and bool(mask.all())
    if not causal and not allones:
        return _numpy_fallback(key, value, query, mask, Wk, bk, Wq, bq, Wv, bv, Wo, bo)

    if causal:
        chunk_ntiles = tuple(4 * (c + 1) for c in range(NMC))
    else:
        chunk_ntiles = tuple(NT for _ in range(NMC))

    bias_qk = bool(np.any(bq) or np.any(bk))
    bias_v = bool(np.any(bv))

    pkey = (chunk_ntiles, causal, bias_qk, bias_v)
    if pkey not in _program_cache:
        _program_cache[pkey] = _build_program(chunk_ntiles, causal, bias_qk, bias_v)
    nc = _program_cache[pkey]

    from concourse.bass_utils import run_bass_kernel_spmd

    in_maps = _host_inputs(key, value, query, Wk, Wq, Wv, Wo, bq, bk, bv, bias_qk, bias_v)
    res = run_bass_kernel_spmd(nc, in_maps, core_ids=list(range(NCORES)))

    outp = np.zeros((B, S, E), np.float32)
    for c in range(NCORES):
        outp[c // G] += res.results[c]["out"]
    outp += bo.astype(np.float32)
    return outp


# revision 21
# speedup vs baseline: 1.1352x; 1.0083x over previous
"""Trainium2 Bass kernel: 16-head MHA (B=2, S=2048, E=1024) on 8 NeuronCores.

Sharding: core c = (batch b = c // 4, head-group g = c % 4); each core runs
4 heads of one batch (data parallel on B x tensor parallel on heads).  The
output projection is row-sharded: each core produces a partial [S, E] f32
output; the host sums the 4 head-group partials per batch and adds bo.

Device schedule: a single chunk-pipelined loop over the four 512-column
m-chunks.  Per chunk c the PE stream is
    proj(c):   k,q chains (weight-stationary, 8 e-tile PSUM chains)
    v(c):      v_aug computed DIRECTLY in [n, dv] layout (x-tile stationary,
               wv moving) -- no PE transposes; interleaved with
    outproj(c-1): per (m-tile, e-half) chains over both pairs
    attn(c):   scoresT (kT stationary, K=64 row-group packed), exp on ACT,
               0/1 triangle multiply on diagonal tiles (gpsimd), attnV
               (v_aug stationary) software-pipelined two steps behind
so the ACT-bound attention of chunk c overlaps the PE-only projection of
chunk c+1 and the output projection of chunk c-1.  The softmax denominator
(from the ones-column of v_aug) takes a single DRAM bounce: written [1,MC],
re-read with a step-0 partition AP as a [64,MC] broadcast, reciprocal via
the fast custom-DVE approx, then one DVE multiply into oT.  Input DMAs are
issued e-tile-granular across the sync/scalar/vector queues in consumption
order so the k-projection starts ~2us in.
"""

import numpy as np
import ml_dtypes

B, S, E = 2, 2048, 1024
H, DK = 16, 64
NCORES = 8
G = 4                 # head-groups (tensor parallel degree)
NH = H // G           # heads per core = 4
DKH = NH * DK         # 256 head dims per core
P = 128
MC = 512              # m-chunk (psum bank width in f32)
NMC = S // MC         # 4 m-chunks
NT = S // P           # 16 n-tiles (and m-tiles)
ET = E // P           # 8 e-tiles
PAIRS = NH // 2       # 2 head pairs per core
BF16 = ml_dtypes.bfloat16
SCALE = float(1.0 / np.sqrt(np.float32(DK)))


def _build_program(chunk_ntiles, causal, bias_qk, bias_v):
    """Build the (SPMD, shared across all 8 cores) Bass program.

    chunk_ntiles[c] = number of 128-wide n-tiles to process for m-chunk c.
    causal: apply diagonal-tile masking (memset + tri multiply).
    """
    from contextlib import ExitStack

    import concourse.bass as bass
    import concourse.tile as tile
    from concourse import bacc, mybir

    f32 = mybir.dt.float32
    bf16 = mybir.dt.bfloat16
    Exp = mybir.ActivationFunctionType.Exp

    nc = bacc.Bacc(
        "TRN2",
        target_bir_lowering=False,
        debug=False,
        enable_asserts=False,
        num_devices=NCORES,
    )

    # ---- DRAM I/O ----
    xqT = nc.dram_tensor("xqT", [E, S], bf16, kind="ExternalInput").ap()
    xkT = nc.dram_tensor("xkT", [E, S], bf16, kind="ExternalInput").ap()
    xvT = nc.dram_tensor("xvT", [E, S], bf16, kind="ExternalInput").ap()
    wkqvT = nc.dram_tensor("wkqvT", [E, 3 * DKH], bf16, kind="ExternalInput").ap()
    woT = nc.dram_tensor("woT", [DKH, E], bf16, kind="ExternalInput").ap()
    dmask = nc.dram_tensor("dmask", [P, P], bf16, kind="ExternalInput").ap()
    if bias_qk:
        bqd = nc.dram_tensor("bq", [DKH, 1], f32, kind="ExternalInput").ap()
        bkd = nc.dram_tensor("bk", [DKH, 1], f32, kind="ExternalInput").ap()
    if bias_v:
        bvd = nc.dram_tensor("bv", [1, DKH], f32, kind="ExternalInput").ap()
    # bf16 partials: halves output DMA + DVE eviction bytes; the host sums
    # the four head-group partials per batch in f32
    out = nc.dram_tensor("out", [S, E], bf16, kind="ExternalOutput").ap()

    with tile.TileContext(nc) as tc, ExitStack() as ctx:
        const = ctx.enter_context(tc.tile_pool(name="const", bufs=1))
        xpool = ctx.enter_context(tc.tile_pool(name="xpool", bufs=1))
        wpool = ctx.enter_context(tc.tile_pool(name="wpool", bufs=1))
        qkpool = ctx.enter_context(tc.tile_pool(name="qkpool", bufs=1))
        vpool = ctx.enter_context(tc.tile_pool(name="vpool", bufs=1))
        prpool = ctx.enter_context(tc.tile_pool(name="prpool", bufs=8))
        rcpool = ctx.enter_context(tc.tile_pool(name="rcpool", bufs=2))
        otpool = ctx.enter_context(tc.tile_pool(name="otpool", bufs=1))
        ostpool = ctx.enter_context(tc.tile_pool(name="ostpool", bufs=4))
        # PSUM: "pj" (proj + outproj chains) 2 banks, "sc" 4 banks,
        # "oaug" 2 banks -- exactly the 8 banks.
        pjps = ctx.enter_context(tc.tile_pool(name="pj_ps", bufs=2, space="PSUM"))
        scps = ctx.enter_context(tc.tile_pool(name="sc_ps", bufs=2, space="PSUM"))
        oaps = ctx.enter_context(tc.tile_pool(name="oa_ps", bufs=2, space="PSUM"))
        rcdram = ctx.enter_context(tc.tile_pool(name="rc_dram", bufs=4, space="DRAM"))

        # ---- persistent SBUF tiles ----
        wkqv_sb = wpool.tile([P, ET, 3 * DKH], bf16, tag="wkqv")
        wo_sb = wpool.tile([P, PAIRS, E], bf16, tag="wo")
        xk_t = xpool.tile([P, ET, S], bf16, tag="xk")
        xq_t = xpool.tile([P, ET, S], bf16, tag="xq")
        xv_t = xpool.tile([P, ET, S], bf16, tag="xv")
        dmask_sb = const.tile([P, P], bf16, tag="dmask")

        # ---- input DMA issue: stripe every tensor across all three DMA
        # queues (sync/scalar HWDGE + gpsimd SWDGE) in consumption order
        # (weights -> xk -> xq -> xv).  A single queue sustains only
        # ~130GB/s, so landing a 4MB tensor fast needs all three.
        def wk_dma(i):
            return (wkqv_sb[:, i, :], wkqvT[P * i : P * (i + 1), :])

        def x_dma(x_sb, xT, i):
            src = xT.rearrange("(t p) s -> p t s", p=P)[:, i, :]
            return (x_sb[:, i, :], src)

        sync_q = []
        gpsimd_q = []
        scalar_q = []
        queues = [sync_q, gpsimd_q, scalar_q]
        for group in (
            [wk_dma(i) for i in range(ET)],
            [x_dma(xk_t, xkT, i) for i in range(ET)],
            [x_dma(xq_t, xqT, i) for i in range(ET)],
            [x_dma(xv_t, xvT, i) for i in range(ET)],
        ):
            for i, tr in enumerate(group):
                queues[i % 3].append(tr)
        # dmask is tiny and first needed at the first diagonal trimask
        # (~35us in); wo at the first woven outproj (~70us in)
        scalar_q.insert(6, (dmask_sb, dmask))
        scalar_q += [
            (wo_sb[:, p, :], woT[P * p : P * (p + 1), :]) for p in range(PAIRS)
        ]
        for eng, q in ((nc.sync, sync_q), (nc.scalar, scalar_q), (nc.gpsimd, gpsimd_q)):
            for dst, src in q:
                eng.dma_start(out=dst, in_=src)

        if bias_qk:
            bq_sb = const.tile([P, PAIRS], f32, tag="bq")
            nc.sync.dma_start(out=bq_sb, in_=bqd.rearrange("(t p) o -> p (t o)", p=P))
            bk_sb = const.tile([P, PAIRS], f32, tag="bk")
            nc.sync.dma_start(out=bk_sb, in_=bkd.rearrange("(t p) o -> p (t o)", p=P))
        if bias_v:
            # bv along the free dim of the direct v_aug layout: broadcast
            # [1, DKH] across all partitions via a step-0 partition AP.
            bv_sb = const.tile([P, DKH], f32, tag="bv")
            nc.sync.dma_start(
                out=bv_sb,
                in_=bass.AP(
                    tensor=bvd.tensor,
                    offset=bvd.offset,
                    ap=[[0, P]] + [list(a) for a in bvd.ap[1:]],
                ),
            )

        # persistent activation tiles
        qT_sb = [qkpool.tile([P, S], bf16, tag=f"qT{p}", name=f"qT_sb{p}") for p in range(PAIRS)]
        kT_sb = [qkpool.tile([P, S], bf16, tag=f"kT{p}", name=f"kT_sb{p}") for p in range(PAIRS)]
        vaug_sb = [vpool.tile([P, NH, DK + 1], bf16, tag=f"va{j}", name=f"vaug_sb{j}") for j in range(NT)]
        oT_sb = [otpool.tile([P, S], bf16, tag=f"oT{p}", name=f"oT_sb{p}") for p in range(PAIRS)]

        wv0 = 2 * DKH

        def proj_chain(dst, w0, x_t, bias, p, c):
            ps = pjps.tile([P, MC], f32, tag="pj", name="ps_kq")
            for i in range(ET):
                nc.tensor.matmul(
                    ps,
                    wkqv_sb[:, i, w0 + P * p : w0 + P * (p + 1)],
                    x_t[:, i, MC * c : MC * (c + 1)],
                    start=(i == 0),
                    stop=(i == ET - 1),
                )
            dslice = dst[p][:, MC * c : MC * (c + 1)]
            if bias is not None:
                nc.vector.tensor_scalar_add(dslice, ps, bias[:, p : p + 1])
            else:
                nc.vector.tensor_copy(dslice, ps)

        def v_chain(j):
            # v_aug[n-tile j] = xv-tile stationary, wv moving: [128 n, 256 dv]
            ps = pjps.tile([P, MC], f32, tag="pj", name="ps_v")
            pv = ps[:, 0:DKH]
            for i in range(ET):
                nc.tensor.matmul(
                    pv,
                    xv_t[:, i, P * j : P * (j + 1)],
                    wkqv_sb[:, i, wv0 : wv0 + DKH],
                    start=(i == 0),
                    stop=(i == ET - 1),
                )
            if bias_v:
                nc.vector.tensor_add(pv, pv, bv_sb)
            pt3 = pv.rearrange("n (h d) -> n h d", h=NH)
            nc.vector.tensor_copy(vaug_sb[j][:, :, 0:DK], pt3)
            nc.gpsimd.memset(vaug_sb[j][:, :, DK : DK + 1], 1.0)

        def op_chain(t, ec, evict_eng, dma_eng):
            op = pjps.tile([P, MC], f32, tag="pj", name="op_t")
            for p in range(PAIRS):
                nc.tensor.matmul(
                    op,
                    oT_sb[p][:, P * t : P * (t + 1)],
                    wo_sb[:, p, MC * ec : MC * (ec + 1)],
                    start=(p == 0),
                    stop=(p == PAIRS - 1),
                )
            ost = ostpool.tile([P, MC], bf16, tag="ost", name="ost_t")
            if evict_eng is nc.scalar:
                nc.scalar.copy(ost, op)
            else:
                evict_eng.tensor_copy(ost, op)
            dma_eng.dma_start(
                out=out[P * t : P * (t + 1), MC * ec : MC * (ec + 1)], in_=ost
            )

        def attn_group(c, p, fillers):
            """fillers: list of zero-arg callables emitting PE filler chains
            (v_aug builds, outproj chains); one is consumed right before
            each attnV step so the PE has work while ACT runs exp."""
            J = chunk_ntiles[c]
            oaug = [
                oaps.tile([P, MC], f32, tag="oaug", name=f"oaug{h01}")
                for h01 in range(2)
            ]
            probs_tiles = [None] * J

            def scores_step(j):
                # columns left of `off` in this m-chunk are fully masked
                # for n-tile j: never compute/exp/consume them
                off = P * (j - 4 * c) if (causal and j >= 4 * c) else 0
                sc = scps.tile([P, 2 * MC], f32, tag="sc", name="sc_ps_t")
                for h01 in range(2):
                    nc.tensor.matmul(
                        sc[:, MC * h01 + off : MC * (h01 + 1)],
                        kT_sb[p][64 * h01 : 64 * (h01 + 1), P * j : P * (j + 1)],
                        qT_sb[p][64 * h01 : 64 * (h01 + 1), MC * c + off : MC * (c + 1)],
                        start=True,
                        stop=True,
                    )
                probs = prpool.tile([P, 2 * MC], bf16, tag="probs", name="probs_t")
                sc3 = sc.rearrange("p (u m) -> p u m", u=2)
                pr3 = probs.rearrange("p (u m) -> p u m", u=2)
                nc.scalar.activation(
                    pr3[:, :, off:MC], sc3[:, :, off:MC], Exp, bias=0.0, scale=SCALE
                )
                if causal and j >= 4 * c:
                    for h01 in range(2):
                        base = MC * h01 + off
                        nc.gpsimd.tensor_mul(
                            probs[:, base : base + P],
                            probs[:, base : base + P],
                            dmask_sb,
                        )
                probs_tiles[j] = (probs, off)

            def attnv_step(j):
                probs, off = probs_tiles[j]
                for h01 in range(2):
                    h = 2 * p + h01
                    nc.tensor.matmul(
                        oaug[h01][0 : DK + 1, off:MC],
                        vaug_sb[j][:, h, :],
                        probs[:, MC * h01 + off : MC * (h01 + 1)],
                        start=(j == 0),
                        stop=(j == J - 1),
                    )

            # software pipeline: scores two steps ahead of attnV, with one
            # PE filler chain in front of each attnV step
            def take_filler():
                if fillers:
                    fillers.pop(0)()

            for j in range(J):
                scores_step(j)
                if j >= 2:
                    take_filler()
                    attnv_step(j - 2)
            take_filler()
            attnv_step(J - 2)
            take_filler()
            attnv_step(J - 1)

            # epilogue: evict o_aug to SBUF (frees PSUM); bounce the
            # denominator row once through DRAM to re-read it broadcast
            # across 64 partitions (step-0 partition AP), fast-reciprocal,
            # multiply into oT.
            osb = []
            for h01 in range(2):
                o = rcpool.tile([DK + 1, MC], f32, tag="osb", bufs=4, name="osb_t")
                nc.vector.tensor_copy(o, oaug[h01][0 : DK + 1, :])
                osb.append(o)
                den_d = rcdram.tile([1, MC], f32, tag="den_d", name="den_d_t")
                nc.sync.dma_start(out=den_d, in_=o[DK : DK + 1, :])
                bcden = rcpool.tile([64, MC], f32, tag="bc", bufs=2, name="bc_t")
                nc.sync.dma_start(
                    out=bcden,
                    in_=bass.AP(
                        tensor=den_d.tensor,
                        offset=den_d.offset,
                        ap=[[0, 64]] + [list(a) for a in den_d.ap[1:]],
                    ),
                )
                osb.append(bcden)
            for h01 in range(2):
                bcden = osb[2 * h01 + 1]
                rc = rcpool.tile([64, MC], f32, tag="rc", bufs=2, name="rc_t")
                nc.vector.reciprocal_approx_fast(out=rc, in_=bcden)
                nc.vector.tensor_mul(
                    oT_sb[p][64 * h01 : 64 * (h01 + 1), MC * c : MC * (c + 1)],
                    osb[2 * h01][0:DK, :],
                    rc,
                )

        # ---- schedule ----
        # All k chains first (paced by the arriving xk stripe), then per
        # chunk: q chains + the two attention groups.  v_aug builds and the
        # previous chunk's outproj chains ride inside the attention groups
        # as PE fillers (one per attnV step), so the exp-paced attention
        # windows keep the PE fed.
        kb = bk_sb if bias_qk else None
        qb = bq_sb if bias_qk else None
        for c in range(NMC):
            for p in range(PAIRS):
                proj_chain(kT_sb, 0, xk_t, kb, p, c)
        for c in range(NMC):
            for p in range(PAIRS):
                proj_chain(qT_sb, DKH, xq_t, qb, p, c)
        for c in range(NMC):
            if causal:
                vjs = list(range(4 * c, 4 * c + 4))
            else:
                vjs = list(range(NT)) if c == 0 else []
            fillers = [
                (lambda j=j: v_chain(j)) for j in vjs
            ]
            if c > 0:
                fillers += [
                    (lambda t=t, ec=ec: op_chain(t, ec, nc.vector, nc.sync))
                    for t in range(4 * (c - 1), 4 * c)
                    for ec in range(E // MC)
                ]
            for p in range(PAIRS):
                attn_group(c, p, fillers)
            for f in fillers:
                f()
            del fillers[:]

        # tail: outproj of the last chunk (ACT is idle now -> scalar evicts)
        n = 0
        for t in range(4 * (NMC - 1), 4 * NMC):
            for ec in range(E // MC):
                op_chain(
                    t,
                    ec,
                    nc.scalar if n % 2 == 0 else nc.vector,
                    nc.sync if n % 2 == 0 else nc.scalar,
                )
                n += 1

    nc.compile()
    return nc


def _host_inputs(key, value, query, Wk, Wq, Wv, Wo, bq, bk, bv, bias_qk, bias_v):
    """Per-core input maps (host-side shard/transpose/cast — not timed)."""
    tri = np.triu(np.ones((P, P), np.float32)).astype(BF16)  # allowed: n<=m
    in_maps = []
    xT = {}
    for b in range(B):
        xT[("q", b)] = np.ascontiguousarray(query[b].T).astype(BF16)
        xT[("k", b)] = np.ascontiguousarray(key[b].T).astype(BF16)
        xT[("v", b)] = np.ascontiguousarray(value[b].T).astype(BF16)
    for c in range(NCORES):
        b, g = divmod(c, G)
        sl = slice(DKH * g, DKH * (g + 1))
        wkqv = np.concatenate(
            [Wk[sl].T, Wq[sl].T, Wv[sl].T], axis=1
        )  # [E, 3*DKH], column blocks K|Q|V
        m = {
            "xqT": xT[("q", b)],
            "xkT": xT[("k", b)],
            "xvT": xT[("v", b)],
            "wkqvT": np.ascontiguousarray(wkqv).astype(BF16),
            "woT": np.ascontiguousarray(Wo[:, sl].T).astype(BF16),
            "dmask": tri,
        }
        if bias_qk:
            m["bq"] = np.ascontiguousarray(bq[sl].astype(np.float32).reshape(DKH, 1))
            m["bk"] = np.ascontiguousarray(bk[sl].astype(np.float32).reshape(DKH, 1))
        if bias_v:
            m["bv"] = np.ascontiguousarray(bv[sl].astype(np.float32).reshape(1, DKH))
        in_maps.append(m)
    return in_maps


def _numpy_fallback(key, value, query, mask, Wk, bk, Wq, bq, Wv, bv, Wo, bo):
    """Exact reference semantics in numpy (general-mask fallback)."""
    def proj(x, W, b):
        return x @ W.T + b

    k = proj(key, Wk, bk).reshape(B, S, H, DK).transpose(0, 2, 1, 3)
    q = proj(query, Wq, bq).reshape(B, S, H, DK).transpose(0, 2, 1, 3)
    v = proj(value, Wv, bv).reshape(B, S, H, DK).transpose(0, 2, 1, 3)
    scores = np.einsum("bhmd,bhnd->bhmn", q, k).astype(np.float32)
    scores = np.where(mask, scores, np.float32(-1e10)) * np.float32(SCALE)
    scores -= scores.max(axis=3, keepdims=True)
    e = np.exp(scores)
    attn = e / e.sum(axis=3, keepdims=True)
    o = np.einsum("bhmn,bhnv->bhmv", attn, v)
    o = o.transpose(0, 2, 1, 3).reshape(B, S, E)
    return (o @ Wo.T + bo).astype(np.float32)


_program_cache = {}


def kernel(key, value, query, mask, Wk, bk, Wq, bq, Wv, bv, Wo, bo):
    key = np.asarray(key, np.float32)
    value = np.asarray(value, np.float32)
    query = np.asarray(query, np.float32)
    mask = np.asarray(mask)
    Wk, bk = np.asarray(Wk, np.float32), np.asarray(bk, np.float32)
    Wq, bq = np.asarray(Wq, np.float32), np.asarray(bq, np.float32)
    Wv, bv = np.asarray(Wv, np.float32), np.asarray(bv, np.float32)
    Wo, bo = np.asarray(Wo, np.float32), np.asarray(bo, np.float32)

    m2 = mask.reshape(B, S, S) if mask.size == B * S * S else None
    causal = m2 is not None and all(
        np.array_equal(m2[b], np.tril(np.ones((S, S), bool))) for b in range(B)
    )
    allones = m2 is not None and bool(mask.all())
    if not causal and not allones:
        return _numpy_fallback(key, value, query, mask, Wk, bk, Wq, bq, Wv, bv, Wo, bo)

    if causal:
        chunk_ntiles = tuple(4 * (c + 1) for c in range(NMC))
    else:
        chunk_ntiles = tuple(NT for _ in range(NMC))

    bias_qk = bool(np.any(bq) or np.any(bk))
    bias_v = bool(np.any(bv))

    pkey = (chunk_ntiles, causal, bias_qk, bias_v)
    if pkey not in _program_cache:
        _program_cache[pkey] = _build_program(chunk_ntiles, causal, bias_qk, bias_v)
    nc = _program_cache[pkey]

    from concourse.bass_utils import run_bass_kernel_spmd

    in_maps = _host_inputs(key, value, query, Wk, Wq, Wv, Wo, bq, bk, bv, bias_qk, bias_v)
    res = run_bass_kernel_spmd(nc, in_maps, core_ids=list(range(NCORES)))

    outp = np.zeros((B, S, E), np.float32)
    for c in range(NCORES):
        outp[c // G] += np.asarray(res.results[c]["out"], np.float32)
    outp += bo.astype(np.float32)
    return outp


# revision 25
# speedup vs baseline: 1.1569x; 1.0192x over previous
"""Trainium2 Bass kernel: 16-head MHA (B=2, S=2048, E=1024) on 8 NeuronCores.

Sharding: core c = (batch b = c // 4, head-group g = c % 4); each core runs
4 heads of one batch (data parallel on B x tensor parallel on heads).  The
output projection is row-sharded: each core produces a partial [S, E] f32
output; the host sums the 4 head-group partials per batch and adds bo.

Device schedule: a single chunk-pipelined loop over the four 512-column
m-chunks.  Per chunk c the PE stream is
    proj(c):   k,q chains (weight-stationary, 8 e-tile PSUM chains)
    v(c):      v_aug computed DIRECTLY in [n, dv] layout (x-tile stationary,
               wv moving) -- no PE transposes; interleaved with
    outproj(c-1): per (m-tile, e-half) chains over both pairs
    attn(c):   scoresT (kT stationary, K=64 row-group packed), exp on ACT,
               0/1 triangle multiply on diagonal tiles (gpsimd), attnV
               (v_aug stationary) software-pipelined two steps behind
so the ACT-bound attention of chunk c overlaps the PE-only projection of
chunk c+1 and the output projection of chunk c-1.  The softmax denominator
(from the ones-column of v_aug) takes a single DRAM bounce: written [1,MC],
re-read with a step-0 partition AP as a [64,MC] broadcast, reciprocal via
the fast custom-DVE approx, then one DVE multiply into oT.  Input DMAs are
issued e-tile-granular across the sync/scalar/vector queues in consumption
order so the k-projection starts ~2us in.
"""

import numpy as np
import ml_dtypes

B, S, E = 2, 2048, 1024
H, DK = 16, 64
NCORES = 8
G = 4                 # head-groups (tensor parallel degree)
NH = H // G           # heads per core = 4
DKH = NH * DK         # 256 head dims per core
P = 128
MC = 512              # m-chunk (psum bank width in f32)
NMC = S // MC         # 4 m-chunks
NT = S // P           # 16 n-tiles (and m-tiles)
ET = E // P           # 8 e-tiles
PAIRS = NH // 2       # 2 head pairs per core
BF16 = ml_dtypes.bfloat16
SCALE = float(1.0 / np.sqrt(np.float32(DK)))


def _build_program(chunk_ntiles, causal, bias_qk, bias_v):
    """Build the (SPMD, shared across all 8 cores) Bass program.

    chunk_ntiles[c] = number of 128-wide n-tiles to process for m-chunk c.
    causal: apply diagonal-tile masking (memset + tri multiply).
    """
    from contextlib import ExitStack

    import concourse.bass as bass
    import concourse.tile as tile
    from concourse import bacc, mybir

    f32 = mybir.dt.float32
    bf16 = mybir.dt.bfloat16
    Exp = mybir.ActivationFunctionType.Exp

    nc = bacc.Bacc(
        "TRN2",
        target_bir_lowering=False,
        debug=False,
        enable_asserts=False,
        num_devices=NCORES,
    )

    # ---- DRAM I/O ----
    # xq blocked by m-chunk, xv blocked by pairs of 128-wide n-tiles: the
    # host pre-permutes so each block is one contiguous-per-partition DMA
    # and lands exactly when the pipeline first needs it.
    xqB = nc.dram_tensor("xqB", [NMC, P, ET * MC], bf16, kind="ExternalInput").ap()
    xkT = nc.dram_tensor("xkT", [E, S], bf16, kind="ExternalInput").ap()
    xvB = nc.dram_tensor("xvB", [ET, P, ET * 2 * P], bf16, kind="ExternalInput").ap()
    wkqvT = nc.dram_tensor("wkqvT", [E, 3 * DKH], bf16, kind="ExternalInput").ap()
    woT = nc.dram_tensor("woT", [DKH, E], bf16, kind="ExternalInput").ap()
    dmask = nc.dram_tensor("dmask", [P, P], bf16, kind="ExternalInput").ap()
    if bias_qk:
        bqd = nc.dram_tensor("bq", [DKH, 1], f32, kind="ExternalInput").ap()
        bkd = nc.dram_tensor("bk", [DKH, 1], f32, kind="ExternalInput").ap()
    if bias_v:
        bvd = nc.dram_tensor("bv", [1, DKH], f32, kind="ExternalInput").ap()
    # bf16 partials: halves output DMA + DVE eviction bytes; the host sums
    # the four head-group partials per batch in f32
    out = nc.dram_tensor("out", [S, E], bf16, kind="ExternalOutput").ap()

    with tile.TileContext(nc) as tc, ExitStack() as ctx:
        const = ctx.enter_context(tc.tile_pool(name="const", bufs=1))
        xpool = ctx.enter_context(tc.tile_pool(name="xpool", bufs=1))
        wpool = ctx.enter_context(tc.tile_pool(name="wpool", bufs=1))
        qkpool = ctx.enter_context(tc.tile_pool(name="qkpool", bufs=1))
        vpool = ctx.enter_context(tc.tile_pool(name="vpool", bufs=1))
        prpool = ctx.enter_context(tc.tile_pool(name="prpool", bufs=8))
        rcpool = ctx.enter_context(tc.tile_pool(name="rcpool", bufs=2))
        otpool = ctx.enter_context(tc.tile_pool(name="otpool", bufs=1))
        ostpool = ctx.enter_context(tc.tile_pool(name="ostpool", bufs=4))
        # PSUM: "pj" (proj + outproj chains) 2 banks, "sc" 4 banks,
        # "oaug" 2 banks -- exactly the 8 banks.
        pjps = ctx.enter_context(tc.tile_pool(name="pj_ps", bufs=2, space="PSUM"))
        scps = ctx.enter_context(tc.tile_pool(name="sc_ps", bufs=2, space="PSUM"))
        oaps = ctx.enter_context(tc.tile_pool(name="oa_ps", bufs=2, space="PSUM"))
        rcdram = ctx.enter_context(tc.tile_pool(name="rc_dram", bufs=4, space="DRAM"))

        # ---- persistent SBUF tiles ----
        wkqv_sb = wpool.tile([P, ET, 3 * DKH], bf16, tag="wkqv")
        wo_sb = wpool.tile([P, PAIRS, E], bf16, tag="wo")
        xk_t = xpool.tile([P, ET, S], bf16, tag="xk")
        xq_t = xpool.tile([P, ET, S], bf16, tag="xq")
        xv_t = xpool.tile([P, ET, S], bf16, tag="xv")
        dmask_sb = const.tile([P, P], bf16, tag="dmask")

        # ---- input DMA issue.  Three concurrent queues (sync/scalar HWDGE
        # + gpsimd SWDGE) each sustain ~HBM/3; transfers are large (fewer
        # per-transfer gaps) and ordered by first consumption:
        # weights -> xk -> xq chunk 0 -> xv n-blocks 0-3 -> xq 1.. -> xv 4..
        def wk_slab(a, b):
            return (wkqv_sb[:, a:b, :], wkqvT[P * a : P * b, :].rearrange(
                "(t p) o -> p t o", p=P))

        def xk_slab(a, b):
            src = xkT.rearrange("(t p) s -> p t s", p=P)[:, a:b, :]
            return (xk_t[:, a:b, :], src)

        def xq_chunk(c):
            src = xqB[c].rearrange("p (t n) -> p t n", t=ET)
            return (xq_t[:, :, MC * c : MC * (c + 1)], src)

        sync_q = [wk_slab(0, 3), xk_slab(0, 3), xq_chunk(0)]
        gpsimd_q = [wk_slab(3, 6), xk_slab(3, 6), xq_chunk(1)]
        scalar_q = [wk_slab(6, 8), xk_slab(6, 8), (dmask_sb, dmask), xq_chunk(2)]
        # xv n-blocks: window c's attnV consumes blocks <= 2c+1
        for nb, q in ((0, sync_q), (1, gpsimd_q), (2, scalar_q), (3, sync_q)):
            src = xvB[nb].rearrange("p (t n) -> p t n", t=ET)
            q.append((xv_t[:, :, 2 * P * nb : 2 * P * (nb + 1)], src))
        gpsimd_q.append(xq_chunk(3))
        for nb, q in ((4, scalar_q), (5, gpsimd_q), (6, sync_q), (7, scalar_q)):
            src = xvB[nb].rearrange("p (t n) -> p t n", t=ET)
            q.append((xv_t[:, :, 2 * P * nb : 2 * P * (nb + 1)], src))
        scalar_q += [
            (wo_sb[:, p, :], woT[P * p : P * (p + 1), :]) for p in range(PAIRS)
        ]
        for eng, q in ((nc.sync, sync_q), (nc.scalar, scalar_q), (nc.gpsimd, gpsimd_q)):
            for dst, src in q:
                eng.dma_start(out=dst, in_=src)

        if bias_qk:
            bq_sb = const.tile([P, PAIRS], f32, tag="bq")
            nc.sync.dma_start(out=bq_sb, in_=bqd.rearrange("(t p) o -> p (t o)", p=P))
            bk_sb = const.tile([P, PAIRS], f32, tag="bk")
            nc.sync.dma_start(out=bk_sb, in_=bkd.rearrange("(t p) o -> p (t o)", p=P))
        if bias_v:
            # bv along the free dim of the direct v_aug layout: broadcast
            # [1, DKH] across all partitions via a step-0 partition AP.
            bv_sb = const.tile([P, DKH], f32, tag="bv")
            nc.sync.dma_start(
                out=bv_sb,
                in_=bass.AP(
                    tensor=bvd.tensor,
                    offset=bvd.offset,
                    ap=[[0, P]] + [list(a) for a in bvd.ap[1:]],
                ),
            )

        # persistent activation tiles
        qT_sb = [qkpool.tile([P, S], bf16, tag=f"qT{p}", name=f"qT_sb{p}") for p in range(PAIRS)]
        kT_sb = [qkpool.tile([P, S], bf16, tag=f"kT{p}", name=f"kT_sb{p}") for p in range(PAIRS)]
        vaug_sb = [vpool.tile([P, NH, DK + 1], bf16, tag=f"va{j}", name=f"vaug_sb{j}") for j in range(NT)]
        oT_sb = [otpool.tile([P, S], bf16, tag=f"oT{p}", name=f"oT_sb{p}") for p in range(PAIRS)]

        wv0 = 2 * DKH

        def proj_chain(dst, w0, x_t, bias, p, c):
            ps = pjps.tile([P, MC], f32, tag="pj", name="ps_kq")
            for i in range(ET):
                nc.tensor.matmul(
                    ps,
                    wkqv_sb[:, i, w0 + P * p : w0 + P * (p + 1)],
                    x_t[:, i, MC * c : MC * (c + 1)],
                    start=(i == 0),
                    stop=(i == ET - 1),
                )
            dslice = dst[p][:, MC * c : MC * (c + 1)]
            if bias is not None:
                nc.vector.tensor_scalar_add(dslice, ps, bias[:, p : p + 1])
            else:
                nc.vector.tensor_copy(dslice, ps)

        def v_chain(j):
            # v_aug[n-tile j] = xv-tile stationary, wv moving: [128 n, 256 dv]
            ps = pjps.tile([P, MC], f32, tag="pj", name="ps_v")
            pv = ps[:, 0:DKH]
            for i in range(ET):
                nc.tensor.matmul(
                    pv,
                    xv_t[:, i, P * j : P * (j + 1)],
                    wkqv_sb[:, i, wv0 : wv0 + DKH],
                    start=(i == 0),
                    stop=(i == ET - 1),
                )
            if bias_v:
                nc.vector.tensor_add(pv, pv, bv_sb)
            pt3 = pv.rearrange("n (h d) -> n h d", h=NH)
            nc.vector.tensor_copy(vaug_sb[j][:, :, 0:DK], pt3)
            nc.gpsimd.memset(vaug_sb[j][:, :, DK : DK + 1], 1.0)

        def op_chain(t, ec, evict_eng, dma_eng):
            op = pjps.tile([P, MC], f32, tag="pj", name="op_t")
            for p in range(PAIRS):
                nc.tensor.matmul(
                    op,
                    oT_sb[p][:, P * t : P * (t + 1)],
                    wo_sb[:, p, MC * ec : MC * (ec + 1)],
                    start=(p == 0),
                    stop=(p == PAIRS - 1),
                )
            ost = ostpool.tile([P, MC], bf16, tag="ost", name="ost_t")
            if evict_eng is nc.scalar:
                nc.scalar.copy(ost, op)
            else:
                evict_eng.tensor_copy(ost, op)
            dma_eng.dma_start(
                out=out[P * t : P * (t + 1), MC * ec : MC * (ec + 1)], in_=ost
            )

        def attn_group(c, p, fillers):
            """fillers: list of zero-arg callables emitting PE filler chains
            (v_aug builds, outproj chains); one is consumed right before
            each attnV step so the PE has work while ACT runs exp."""
            J = chunk_ntiles[c]
            oaug = [
                oaps.tile([P, MC], f32, tag="oaug", name=f"oaug{h01}")
                for h01 in range(2)
            ]
            probs_tiles = [None] * J

            def scores_step(j):
                # columns left of `off` in this m-chunk are fully masked
                # for n-tile j: never compute/exp/consume them
                off = P * (j - 4 * c) if (causal and j >= 4 * c) else 0
                sc = scps.tile([P, 2 * MC], f32, tag="sc", name="sc_ps_t")
                for h01 in range(2):
                    nc.tensor.matmul(
                        sc[:, MC * h01 + off : MC * (h01 + 1)],
                        kT_sb[p][64 * h01 : 64 * (h01 + 1), P * j : P * (j + 1)],
                        qT_sb[p][64 * h01 : 64 * (h01 + 1), MC * c + off : MC * (c + 1)],
                        start=True,
                        stop=True,
                    )
                probs = prpool.tile([P, 2 * MC], bf16, tag="probs", name="probs_t")
                sc3 = sc.rearrange("p (u m) -> p u m", u=2)
                pr3 = probs.rearrange("p (u m) -> p u m", u=2)
                nc.scalar.activation(
                    pr3[:, :, off:MC], sc3[:, :, off:MC], Exp, bias=0.0, scale=SCALE
                )
                if causal and j >= 4 * c:
                    for h01 in range(2):
                        base = MC * h01 + off
                        nc.gpsimd.tensor_mul(
                            probs[:, base : base + P],
                            probs[:, base : base + P],
                            dmask_sb,
                        )
                probs_tiles[j] = (probs, off)

            def attnv_step(j):
                probs, off = probs_tiles[j]
                for h01 in range(2):
                    h = 2 * p + h01
                    nc.tensor.matmul(
                        oaug[h01][0 : DK + 1, off:MC],
                        vaug_sb[j][:, h, :],
                        probs[:, MC * h01 + off : MC * (h01 + 1)],
                        start=(j == 0),
                        stop=(j == J - 1),
                    )

            # software pipeline: scores two steps ahead of attnV, with one
            # PE filler chain in front of each attnV step
            def take_filler():
                if fillers:
                    fillers.pop(0)()

            for j in range(J):
                scores_step(j)
                if j >= 2:
                    take_filler()
                    attnv_step(j - 2)
            take_filler()
            attnv_step(J - 2)
            take_filler()
            attnv_step(J - 1)

            # epilogue: evict o_aug to SBUF (frees PSUM); bounce the
            # denominator row once through DRAM to re-read it broadcast
            # across 64 partitions (step-0 partition AP), fast-reciprocal,
            # multiply into oT.
            osb = []
            for h01 in range(2):
                o = rcpool.tile([DK + 1, MC], f32, tag="osb", bufs=4, name="osb_t")
                nc.vector.tensor_copy(o, oaug[h01][0 : DK + 1, :])
                osb.append(o)
                den_d = rcdram.tile([1, MC], f32, tag="den_d", name="den_d_t")
                nc.sync.dma_start(out=den_d, in_=o[DK : DK + 1, :])
                bcden = rcpool.tile([64, MC], f32, tag="bc", bufs=2, name="bc_t")
                nc.sync.dma_start(
                    out=bcden,
                    in_=bass.AP(
                        tensor=den_d.tensor,
                        offset=den_d.offset,
                        ap=[[0, 64]] + [list(a) for a in den_d.ap[1:]],
                    ),
                )
                osb.append(bcden)
            for h01 in range(2):
                bcden = osb[2 * h01 + 1]
                rc = rcpool.tile([64, MC], f32, tag="rc", bufs=2, name="rc_t")
                nc.vector.reciprocal_approx_fast(out=rc, in_=bcden)
                nc.vector.tensor_mul(
                    oT_sb[p][64 * h01 : 64 * (h01 + 1), MC * c : MC * (c + 1)],
                    osb[2 * h01][0:DK, :],
                    rc,
                )

        # ---- schedule ----
        # All k chains first (paced by the arriving xk stripe), then per
        # chunk: q chains + the two attention groups.  v_aug builds and the
        # previous chunk's outproj chains ride inside the attention groups
        # as PE fillers (one per attnV step), so the exp-paced attention
        # windows keep the PE fed.
        kb = bk_sb if bias_qk else None
        qb = bq_sb if bias_qk else None
        for c in range(NMC):
            for p in range(PAIRS):
                proj_chain(kT_sb, 0, xk_t, kb, p, c)
        for c in range(NMC):
            for p in range(PAIRS):
                proj_chain(qT_sb, DKH, xq_t, qb, p, c)
        for c in range(NMC):
            if causal:
                vjs = list(range(4 * c, 4 * c + 4))
            else:
                vjs = list(range(NT)) if c == 0 else []
            fillers = [
                (lambda j=j: v_chain(j)) for j in vjs
            ]
            if c > 0:
                fillers += [
                    (lambda t=t, ec=ec: op_chain(t, ec, nc.vector, nc.sync))
                    for t in range(4 * (c - 1), 4 * c)
                    for ec in range(E // MC)
                ]
            for p in range(PAIRS):
                attn_group(c, p, fillers)
            for f in fillers:
                f()
            del fillers[:]

        # tail: outproj of the last chunk (ACT is idle now -> scalar evicts)
        n = 0
        for t in range(4 * (NMC - 1), 4 * NMC):
            for ec in range(E // MC):
                op_chain(
                    t,
                    ec,
                    nc.scalar if n % 2 == 0 else nc.vector,
                    nc.sync if n % 2 == 0 else nc.scalar,
                )
                n += 1

    nc.compile()
    return nc


def _host_inputs(key, value, query, Wk, Wq, Wv, Wo, bq, bk, bv, bias_qk, bias_v):
    """Per-core input maps (host-side shard/transpose/cast — not timed)."""
    tri = np.triu(np.ones((P, P), np.float32)).astype(BF16)  # allowed: n<=m
    in_maps = []
    xT = {}
    for b in range(B):
        xqT = query[b].T.astype(BF16)  # [E, S]
        xkT = np.ascontiguousarray(key[b].T).astype(BF16)
        xvT = value[b].T.astype(BF16)
        # xqB[c, p, t*MC+n] = xqT[t*P+p, c*MC+n]  (m-chunk blocked)
        xT[("qB", b)] = np.ascontiguousarray(
            xqT.reshape(ET, P, NMC, MC).transpose(2, 1, 0, 3).reshape(NMC, P, ET * MC)
        )
        # xvB[nb, p, t*256+n] = xvT[t*P+p, nb*256+n]  (n-block blocked)
        xT[("vB", b)] = np.ascontiguousarray(
            xvT.reshape(ET, P, ET, 2 * P).transpose(2, 1, 0, 3).reshape(ET, P, ET * 2 * P)
        )
        xT[("k", b)] = xkT
    for c in range(NCORES):
        b, g = divmod(c, G)
        sl = slice(DKH * g, DKH * (g + 1))
        wkqv = np.concatenate(
            [Wk[sl].T, Wq[sl].T, Wv[sl].T], axis=1
        )  # [E, 3*DKH], column blocks K|Q|V
        m = {
            "xqB": xT[("qB", b)],
            "xkT": xT[("k", b)],
            "xvB": xT[("vB", b)],
            "wkqvT": np.ascontiguousarray(wkqv).astype(BF16),
            "woT": np.ascontiguousarray(Wo[:, sl].T).astype(BF16),
            "dmask": tri,
        }
        if bias_qk:
            m["bq"] = np.ascontiguousarray(bq[sl].astype(np.float32).reshape(DKH, 1))
            m["bk"] = np.ascontiguousarray(bk[sl].astype(np.float32).reshape(DKH, 1))
        if bias_v:
            m["bv"] = np.ascontiguousarray(bv[sl].astype(np.float32).reshape(1, DKH))
        in_maps.append(m)
    return in_maps


def _numpy_fallback(key, value, query, mask, Wk, bk, Wq, bq, Wv, bv, Wo, bo):
    """Exact reference semantics in numpy (general-mask fallback)."""
    def proj(x, W, b):
        return x @ W.T + b

    k = proj(key, Wk, bk).reshape(B, S, H, DK).transpose(0, 2, 1, 3)
    q = proj(query, Wq, bq).reshape(B, S, H, DK).transpose(0, 2, 1, 3)
    v = proj(value, Wv, bv).reshape(B, S, H, DK).transpose(0, 2, 1, 3)
    scores = np.einsum("bhmd,bhnd->bhmn", q, k).astype(np.float32)
    scores = np.where(mask, scores, np.float32(-1e10)) * np.float32(SCALE)
    scores -= scores.max(axis=3, keepdims=True)
    e = np.exp(scores)
    attn = e / e.sum(axis=3, keepdims=True)
    o = np.einsum("bhmn,bhnv->bhmv", attn, v)
    o = o.transpose(0, 2, 1, 3).reshape(B, S, E)
    return (o @ Wo.T + bo).astype(np.float32)


_program_cache = {}


def kernel(key, value, query, mask, Wk, bk, Wq, bq, Wv, bv, Wo, bo):
    key = np.asarray(key, np.float32)
    value = np.asarray(value, np.float32)
    query = np.asarray(query, np.float32)
    mask = np.asarray(mask)
    Wk, bk = np.asarray(Wk, np.float32), np.asarray(bk, np.float32)
    Wq, bq = np.asarray(Wq, np.float32), np.asarray(bq, np.float32)
    Wv, bv = np.asarray(Wv, np.float32), np.asarray(bv, np.float32)
    Wo, bo = np.asarray(Wo, np.float32), np.asarray(bo, np.float32)

    m2 = mask.reshape(B, S, S) if mask.size == B * S * S else None
    causal = m2 is not None and all(
        np.array_equal(m2[b], np.tril(np.ones((S, S), bool))) for b in range(B)
    )
    allones = m2 is not None and bool(mask.all())
    if not causal and not allones:
        return _numpy_fallback(key, value, query, mask, Wk, bk, Wq, bq, Wv, bv, Wo, bo)

    if causal:
        chunk_ntiles = tuple(4 * (c + 1) for c in range(NMC))
    else:
        chunk_ntiles = tuple(NT for _ in range(NMC))

    bias_qk = bool(np.any(bq) or np.any(bk))
    bias_v = bool(np.any(bv))

    pkey = (chunk_ntiles, causal, bias_qk, bias_v)
    if pkey not in _program_cache:
        _program_cache[pkey] = _build_program(chunk_ntiles, causal, bias_qk, bias_v)
    nc = _program_cache[pkey]

    from concourse.bass_utils import run_bass_kernel_spmd

    in_maps = _host_inputs(key, value, query, Wk, Wq, Wv, Wo, bq, bk, bv, bias_qk, bias_v)
    res = run_bass_kernel_spmd(nc, in_maps, core_ids=list(range(NCORES)))

    outp = np.zeros((B, S, E), np.float32)
    for c in range(NCORES):
        outp[c // G] += np.asarray(res.results[c]["out"], np.float32)
    outp += bo.astype(np.float32)
    return outp
